# revision 15
# baseline (speedup 1.0000x reference)
"""GQA causal attention block (sparse_attention) on 8 Trainium2 NeuronCores.

Tensor-parallel over heads: core i computes q-heads 4i..4i+3 and kv-head i
(N_KV == n_cores, so each core owns exactly one kv head), plus the matching
row-slice of the o_proj; the 8 partial o_proj outputs are summed on the host.

Layout choice: everything that feeds the PE keeps the contraction dim on
partitions. Projections produce qT/kT/vT [d, s] directly (stationary = weight
chunk, moving = xT), attention scores are computed transposed [t, s]
(stationary = kT slice, moving = qT), PV consumes v [t, d] (stationary) times
exp-scores [t, s] (moving), and o_proj consumes outT [d, s] as stationary.
Softmax denominators come from a ones-matmul (partition-dim reduction on PE,
result pre-broadcast across partitions); reciprocals/rsqrts are computed as
exp(-ln(x)) on the ACT engine to avoid the slow iterative DVE divide.
"""

import sys

sys.path.insert(0, "/opt/trn_rl_repo")

import numpy as np
import ml_dtypes

import concourse.bass as bass
import concourse.mybir as mybir
from concourse import tile
from concourse.vector_clock import ScopedClock, VectorClock
from concourse.bass_utils import run_bass_kernel_spmd

F32 = mybir.dt.float32
BF16 = mybir.dt.bfloat16
AF = mybir.ActivationFunctionType
OP = mybir.AluOpType

S = 2048
HID = 4096
N_HEADS = 32
N_KV = 8
D = 128
NCORES = 8
QH = N_HEADS // NCORES          # q heads per core
EPS = 1e-6
SM_SCALE = float(D) ** -0.5
NJ = S // 512                   # 512-wide s blocks
NHC = HID // 128                # 128-deep contraction chunks
NT = S // 128                   # 128-tall t tiles


class TileContextFixed(tile.TileContext):
    """TileContext whose tail drain emits one sem-wait per Drain instruction.

    The pinned walrus (CoreV3GenImpl setupSyncWait) rejects instructions that
    carry more than one sync-wait command; stock TileContext attaches the
    whole global clock to a single Drain.
    """

    def _drain_and_barrier(self, tick_clock, wait_clock):
        gc = tick_clock.global_clock
        nprocs = len(gc)
        emitted = False
        for proc in range(nprocs):
            tick = gc[proc]
            if tick <= 0:
                continue
            vec = [0] * nprocs
            vec[proc] = tick
            d = self.nc.sync.drain()
            wait_clock.add_sem_waits(d.ins, ScopedClock({None: VectorClock(vec)}))
            emitted = True
        if not emitted:
            self.nc.sync.drain()

        self.nc.all_engine_barrier()
        assert self.sems is not None
        popped = self.nc._tile_sem_poison_stack.pop()
        assert popped is self._sem_poison
        self.nc.clear_and_free_semaphores(list(self.sems.allocated().values()))
        self.nc.all_engine_barrier()


def _split_multi_waits(nc):
    """Hoist all-but-one sem wait of any instruction onto preceding NOPs.

    The pinned walrus rejects instructions with more than one sync-wait
    command; engine streams execute in order, so a same-engine NOP carrying
    the extra waits right before the instruction is equivalent.
    """
    n = 0
    for f in nc.m.functions:
        for bb in f.blocks:
            rebuilt = []
            changed = False
            for inst in bb.instructions:
                si = inst.sync_info
                if si is not None and len(si.on_wait) > 1:
                    waits = list(si.on_wait)
                    for w in waits[:-1]:
                        n += 1
                        nop = mybir.InstNoOp(
                            name=f"I-waitsplit-{n}",
                            engine=inst.engine,
                            sync_info=mybir.SyncInfo(on_wait=[w], on_update=[]),
                            bass_nofuse=True,
                        )
                        nc.register_instruction(nop)
                        rebuilt.append(nop)
                    inst.sync_info = mybir.SyncInfo(
                        on_wait=[waits[-1]], on_update=list(si.on_update)
                    )
                    changed = True
                rebuilt.append(inst)
            if changed:
                bb.instructions = rebuilt


def build_program():
    nc = bass.Bass()

    xt = nc.dram_tensor("xt", [HID, S], BF16, kind="ExternalInput")
    # packed per-core projection weights: [HID, 4*D q | D k | D v]
    wqkv = nc.dram_tensor("wqkv", [HID, (QH + 2) * D], BF16, kind="ExternalInput")
    wo = nc.dram_tensor("wo", [QH * D, HID], BF16, kind="ExternalInput")
    # packed rope tables: [:, 0, :] = cos*w; [:, 1, :] = half-swapped rotate
    # table swS with swS[d] = sign(pair(d))*sin[pair(d)]*w[d], so that
    # rot-half multiplies read both SBUF operands at the same base partition
    tabq = nc.dram_tensor("tabq", [D, 2, S], F32, kind="ExternalInput")
    tabk = nc.dram_tensor("tabk", [D, 2, S], F32, kind="ExternalInput")
    maskt = nc.dram_tensor("maskt", [16, D, 512], F32, kind="ExternalInput")
    identb = nc.dram_tensor("identb", [D, D], BF16, kind="ExternalInput")
    out = nc.dram_tensor("out", [S, HID], F32, kind="ExternalOutput")

    with TileContextFixed(nc) as tc:
        with (
            tc.tile_pool(name="const", bufs=1) as constp,
            tc.tile_pool(name="persist", bufs=1) as persist,
            tc.tile_pool(name="wstream", bufs=8) as wstream,
            tc.tile_pool(name="xstream", bufs=8) as xstream,
            tc.tile_pool(name="tmp", bufs=2) as tmp,
            tc.tile_pool(name="tabstream", bufs=4) as tabstream,
            tc.tile_pool(name="expp", bufs=4) as expp,
            tc.tile_pool(name="outsb", bufs=2) as outsb,
            tc.tile_pool(name="ps", bufs=8, space="PSUM") as ps,
        ):
            ident = constp.tile([D, D], BF16, tag="ident")
            nc.gpsimd.dma_start(ident[:], identb[:])
            ones = constp.tile([D, D], BF16, tag="ones")
            nc.vector.memset(ones[:], 1.0)
            onesf = constp.tile([D, D], F32, tag="onesf")
            nc.vector.memset(onesf[:], 1.0)
            epsb = constp.tile([D, 1], F32, tag="epsb")
            nc.vector.memset(epsb[:], EPS)

            masks = persist.tile([D, 16, 512], F32, tag="masks")
            nc.gpsimd.dma_start(masks[:], maskt[:].rearrange("t p f -> p t f"))
            wosb = persist.tile([D, QH, HID], BF16, tag="wosb")
            nc.gpsimd.dma_start(wosb[:], wo[:].rearrange("(h p) f -> p h f", p=D))

            qhat = [persist.tile([D, S], BF16, tag=f"qhat{h}", name=f"qhat{h}")
                    for h in range(QH)]
            khat = persist.tile([D, S], BF16, tag="khat")
            vsb = persist.tile([D, NT, D], BF16, tag="vsb")
            outt = [persist.tile([D, S], BF16, tag=f"outt{h}", name=f"outt{h}")
                    for h in range(QH)]

            def emit_proj_rope(j):
                """Projections + rms-norm + rope + v transpose for s block j."""
                js = slice(512 * j, 512 * (j + 1))
                pq = [ps.tile([D, 512], F32, tag="ps", name=f"pq{_h}")
                      for _h in range(QH)]
                pk = ps.tile([D, 512], F32, tag="ps", name="pk")
                pv = ps.tile([D, 512], F32, tag="ps", name="pv")
                for hc in range(NHC):
                    xt_t = xstream.tile([D, 512], BF16, tag="xt", name="xt_t")
                    nc.sync.dma_start(xt_t[:], xt[128 * hc:128 * (hc + 1), js])
                    w_t = wstream.tile([D, (QH + 2) * D], BF16, tag="w", name="w_t")
                    nc.gpsimd.dma_start(w_t[:], wqkv[128 * hc:128 * (hc + 1), :])
                    st = dict(start=(hc == 0), stop=(hc == NHC - 1))
                    for h in range(QH):
                        nc.tensor.matmul(pq[h][:], w_t[:, 128 * h:128 * (h + 1)],
                                         xt_t[:], **st)
                    nc.tensor.matmul(pk[:], w_t[:, QH * D:(QH + 1) * D], xt_t[:], **st)
                    nc.tensor.matmul(pv[:], w_t[:, (QH + 1) * D:], xt_t[:], **st)

                # k first so attention on this block can start earliest
                for h in [QH] + list(range(QH)):
                    if h < QH:
                        psrc, dstt, tdram = pq[h], qhat[h], tabq
                    else:
                        psrc, dstt, tdram = pk, khat, tabk
                    tab = tabstream.tile([D, 2, 512], F32, tag="tab", name="tab")
                    nc.sync.dma_start(tab[:], tdram[:, :, js])
                    # single eviction read frees the PSUM bank quickly
                    qraw = tmp.tile([D, 512], F32, tag="qraw", bufs=3, name="qraw")
                    nc.scalar.copy(qraw[:], psrc[:])
                    sq = tmp.tile([D, 512], BF16, tag="sq", name="sq")
                    nc.scalar.square(sq[:], qraw[:])
                    pss = ps.tile([D, 512], F32, tag="ps", name="pss")
                    nc.tensor.matmul(pss[:], ones[:], sq[:], start=True, stop=True)
                    # r = rsqrt(mean + eps) = exp(-0.5 * ln(sumsq/128 + eps))
                    rbc = tmp.tile([D, 512], F32, tag="rbc", name="rbc")
                    nc.scalar.activation(rbc[:], pss[:], AF.Ln,
                                         bias=epsb[:], scale=1.0 / D)
                    nc.scalar.activation(rbc[:], rbc[:], AF.Exp, bias=0.0, scale=-0.5)
                    t1 = tmp.tile([D, 512], F32, tag="t1", name="t1")
                    nc.vector.tensor_tensor(t1[:], qraw[:], tab[:, 0, :], OP.mult)
                    t2 = tmp.tile([D, 512], F32, tag="t2", name="t2")
                    nc.vector.tensor_tensor(t2[0:64, :], qraw[64:128, :],
                                            tab[64:128, 1, :], OP.mult)
                    nc.vector.tensor_tensor(t2[64:128, :], qraw[0:64, :],
                                            tab[0:64, 1, :], OP.mult)
                    nc.vector.tensor_tensor(t1[:], t1[:], t2[:], OP.add)
                    nc.vector.tensor_tensor(dstt[:, js], t1[:], rbc[:], OP.mult)

                # v: evict + transpose into [t, d] layout
                vt = tmp.tile([D, 512], BF16, tag="vt", name="vt")
                nc.vector.tensor_copy(vt[:], pv[:])
                for c in range(4):
                    pvt = ps.tile([D, D], BF16, tag="ps", name="pvt")
                    nc.tensor.transpose(pvt[:], vt[:, 128 * c:128 * (c + 1)], ident[:])
                    nc.scalar.copy(vsb[:, 4 * j + c, :], pvt[:])

            def emit_attention(j):
                """Attention + o_proj for s block j (k/v tiles 0..4j+3 ready)."""
                js = slice(512 * j, 512 * (j + 1))
                for h in range(QH):
                    po = ps.tile([D, 512], F32, tag="ps", name="po")
                    pd = ps.tile([D, 512], F32, tag="ps", name="pd")
                    ntt = 4 * j + 4
                    for tt in range(ntt):
                        psc = ps.tile([D, 512], F32, tag="ps", name="psc")
                        nc.tensor.matmul(psc[:], khat[:, 128 * tt:128 * (tt + 1)],
                                         qhat[h][:, js], start=True, stop=True)
                        if tt >= 4 * j:
                            nc.vector.tensor_tensor(psc[:], psc[:], masks[:, tt, :],
                                                    OP.add)
                        ex = expp.tile([D, 512], BF16, tag="ex", name="ex")
                        nc.scalar.activation(ex[:], psc[:], AF.Exp,
                                             bias=0.0, scale=SM_SCALE)
                        st = dict(start=(tt == 0), stop=(tt == ntt - 1))
                        nc.tensor.matmul(po[:], vsb[:, tt, :], ex[:], **st)
                        nc.tensor.matmul(pd[:], ones[:], ex[:], **st)
                    rd = tmp.tile([D, 512], F32, tag="rd", name="rd")
                    nc.scalar.activation(rd[:], pd[:], AF.Ln, bias=0.0, scale=1.0)
                    nc.scalar.activation(rd[:], rd[:], AF.Exp, bias=0.0, scale=-1.0)
                    nc.vector.tensor_tensor(outt[h][:, js], po[:], rd[:], OP.mult)

                for stt in range(4 * j, 4 * j + 4):
                    ss = slice(128 * stt, 128 * (stt + 1))
                    for half in range(2):
                        pb = [ps.tile([D, 512], F32, tag="ps", name=f"pb{_b}")
                              for _b in range(4)]
                        for h in range(QH):
                            for b in range(4):
                                col = 2048 * half + 512 * b
                                nc.tensor.matmul(pb[b][:], outt[h][:, ss],
                                                 wosb[:, h, col:col + 512],
                                                 start=(h == 0), stop=(h == QH - 1))
                        osb = outsb.tile([D, 2048], F32, tag="osb", name="osb")
                        for b in range(4):
                            nc.scalar.copy(osb[:, 512 * b:512 * (b + 1)], pb[b][:])
                        nc.gpsimd.dma_start(out[ss, 2048 * half:2048 * (half + 1)],
                                            osb[:])

            # Software-pipeline by one block: attention(j-1) is emitted after
            # proj(j), so PE always has projection matmuls to run while the
            # rope chain for block j drains on ACT/DVE.
            for j in range(NJ):
                emit_proj_rope(j)
                if j > 0:
                    emit_attention(j - 1)
            emit_attention(NJ - 1)

    _split_multi_waits(nc)
    return nc


_NC_CACHE = None


def _get_program():
    global _NC_CACHE
    if _NC_CACHE is None:
        _NC_CACHE = build_program()
    return _NC_CACHE


def _rope_tables(cos_g, sin_g, w):
    """Pack [D, 2, S]: [:, 0] = cos_g.T * w[d]; [:, 1] = swS where
    swS[d, s] = sign(pair(d)) * sin_g[s, pair(d)] * w[d], i.e. the rotate
    table with halves pre-swapped so t2[lo] = qraw[hi] * swS[hi] etc."""
    half = D // 2
    cw = np.ascontiguousarray((cos_g * w[None, :]).T)
    swS = np.empty((D, S), np.float32)
    swS[:half, :] = (sin_g[:, half:] * w[:half][None, :]).T
    swS[half:, :] = -(sin_g[:, :half] * w[half:][None, :]).T
    return np.ascontiguousarray(np.stack([cw, swS], axis=1))  # [D, 2, S]


def kernel(x, position_ids, cos, sin, attn_mask, Wq, Wk, Wv, Wo, q_norm_w, k_norm_w):
    x = np.asarray(x, np.float32)
    position_ids = np.asarray(position_ids)
    cos_g = np.asarray(cos, np.float32)[position_ids]   # [S, D]
    sin_g = np.asarray(sin, np.float32)[position_ids]
    attn_mask = np.asarray(attn_mask, np.float32)
    Wq = np.asarray(Wq, np.float32)
    Wk = np.asarray(Wk, np.float32)
    Wv = np.asarray(Wv, np.float32)
    Wo = np.asarray(Wo, np.float32)
    qw = np.asarray(q_norm_w, np.float32)
    kw = np.asarray(k_norm_w, np.float32)

    bf = ml_dtypes.bfloat16
    xt = np.ascontiguousarray(x.T).astype(bf)           # [HID, S]

    tabq = _rope_tables(cos_g, sin_g, qw)
    tabk = _rope_tables(cos_g, sin_g, kw)

    # diagonal-band mask tiles of attn_mask.T: tile tt covers scoresT rows
    # 128*tt..128*tt+127 and cols (q positions) 512*(tt//4)..+511
    mT = attn_mask.T
    maskt = np.empty((16, D, 512), np.float32)
    for tt in range(16):
        j = tt // 4
        maskt[tt] = mT[128 * tt:128 * (tt + 1), 512 * j:512 * (j + 1)]

    identb = np.eye(D).astype(bf)

    in_maps = []
    for i in range(NCORES):
        wqkv = np.concatenate([
            Wq[:, QH * D * i:QH * D * (i + 1)],
            Wk[:, D * i:D * (i + 1)],
            Wv[:, D * i:D * (i + 1)],
        ], axis=1).astype(bf)
        in_maps.append({
            "xt": xt,
            "wqkv": np.ascontiguousarray(wqkv),
            "wo": np.ascontiguousarray(Wo[QH * D * i:QH * D * (i + 1), :]).astype(bf),
            "tabq": tabq, "tabk": tabk,
            "maskt": maskt,
            "identb": identb,
        })

    nc = _get_program()
    res = run_bass_kernel_spmd(nc, in_maps, list(range(NCORES)))
    acc = np.zeros((S, HID), np.float32)
    for r in res.results:
        acc += r["out"]
    return acc


# revision 16
# speedup vs baseline: 1.0018x; 1.0018x over previous
"""GQA causal attention block (sparse_attention) on 8 Trainium2 NeuronCores.

Tensor-parallel over heads: core i computes q-heads 4i..4i+3 and kv-head i
(N_KV == n_cores, so each core owns exactly one kv head), plus the matching
row-slice of the o_proj; the 8 partial o_proj outputs are summed on the host.

Layout choice: everything that feeds the PE keeps the contraction dim on
partitions. Projections produce qT/kT/vT [d, s] directly (stationary = weight
chunk, moving = xT), attention scores are computed transposed [t, s]
(stationary = kT slice, moving = qT), PV consumes v [t, d] (stationary) times
exp-scores [t, s] (moving), and o_proj consumes outT [d, s] as stationary.
Softmax denominators come from a ones-matmul (partition-dim reduction on PE,
result pre-broadcast across partitions); reciprocals/rsqrts are computed as
exp(-ln(x)) on the ACT engine to avoid the slow iterative DVE divide.
"""

import sys

sys.path.insert(0, "/opt/trn_rl_repo")

import numpy as np
import ml_dtypes

import concourse.bass as bass
import concourse.mybir as mybir
from concourse import tile
from concourse.vector_clock import ScopedClock, VectorClock
from concourse.bass_utils import run_bass_kernel_spmd

F32 = mybir.dt.float32
BF16 = mybir.dt.bfloat16
AF = mybir.ActivationFunctionType
OP = mybir.AluOpType

S = 2048
HID = 4096
N_HEADS = 32
N_KV = 8
D = 128
NCORES = 8
QH = N_HEADS // NCORES          # q heads per core
EPS = 1e-6
SM_SCALE = float(D) ** -0.5
NJ = S // 512                   # 512-wide s blocks
NHC = HID // 128                # 128-deep contraction chunks
NT = S // 128                   # 128-tall t tiles


class TileContextFixed(tile.TileContext):
    """TileContext whose tail drain emits one sem-wait per Drain instruction.

    The pinned walrus (CoreV3GenImpl setupSyncWait) rejects instructions that
    carry more than one sync-wait command; stock TileContext attaches the
    whole global clock to a single Drain.
    """

    def _drain_and_barrier(self, tick_clock, wait_clock):
        gc = tick_clock.global_clock
        nprocs = len(gc)
        emitted = False
        for proc in range(nprocs):
            tick = gc[proc]
            if tick <= 0:
                continue
            vec = [0] * nprocs
            vec[proc] = tick
            d = self.nc.sync.drain()
            wait_clock.add_sem_waits(d.ins, ScopedClock({None: VectorClock(vec)}))
            emitted = True
        if not emitted:
            self.nc.sync.drain()

        self.nc.all_engine_barrier()
        assert self.sems is not None
        popped = self.nc._tile_sem_poison_stack.pop()
        assert popped is self._sem_poison
        self.nc.clear_and_free_semaphores(list(self.sems.allocated().values()))
        self.nc.all_engine_barrier()


def _split_multi_waits(nc):
    """Hoist all-but-one sem wait of any instruction onto preceding NOPs.

    The pinned walrus rejects instructions with more than one sync-wait
    command; engine streams execute in order, so a same-engine NOP carrying
    the extra waits right before the instruction is equivalent.
    """
    n = 0
    for f in nc.m.functions:
        for bb in f.blocks:
            rebuilt = []
            changed = False
            for inst in bb.instructions:
                si = inst.sync_info
                if si is not None and len(si.on_wait) > 1:
                    waits = list(si.on_wait)
                    for w in waits[:-1]:
                        n += 1
                        nop = mybir.InstNoOp(
                            name=f"I-waitsplit-{n}",
                            engine=inst.engine,
                            sync_info=mybir.SyncInfo(on_wait=[w], on_update=[]),
                            bass_nofuse=True,
                        )
                        nc.register_instruction(nop)
                        rebuilt.append(nop)
                    inst.sync_info = mybir.SyncInfo(
                        on_wait=[waits[-1]], on_update=list(si.on_update)
                    )
                    changed = True
                rebuilt.append(inst)
            if changed:
                bb.instructions = rebuilt


def build_program():
    nc = bass.Bass()

    xt = nc.dram_tensor("xt", [HID, S], BF16, kind="ExternalInput")
    # packed per-core projection weights: [HID, 4*D q | D k | D v]
    wqkv = nc.dram_tensor("wqkv", [HID, (QH + 2) * D], BF16, kind="ExternalInput")
    wo = nc.dram_tensor("wo", [QH * D, HID], BF16, kind="ExternalInput")
    # packed rope tables: [:, 0, :] = cos*w; [:, 1, :] = half-swapped rotate
    # table swS with swS[d] = sign(pair(d))*sin[pair(d)]*w[d], so that
    # rot-half multiplies read both SBUF operands at the same base partition
    tabq = nc.dram_tensor("tabq", [D, 2, S], F32, kind="ExternalInput")
    tabk = nc.dram_tensor("tabk", [D, 2, S], F32, kind="ExternalInput")
    maskt = nc.dram_tensor("maskt", [16, D, 512], BF16, kind="ExternalInput")
    identb = nc.dram_tensor("identb", [D, D], BF16, kind="ExternalInput")
    out = nc.dram_tensor("out", [S, HID], F32, kind="ExternalOutput")

    with TileContextFixed(nc) as tc:
        with (
            tc.tile_pool(name="const", bufs=1) as constp,
            tc.tile_pool(name="persist", bufs=1) as persist,
            tc.tile_pool(name="wstream", bufs=8) as wstream,
            tc.tile_pool(name="xstream", bufs=8) as xstream,
            tc.tile_pool(name="tmp", bufs=2) as tmp,
            tc.tile_pool(name="tabstream", bufs=4) as tabstream,
            tc.tile_pool(name="expp", bufs=4) as expp,
            tc.tile_pool(name="outsb", bufs=2) as outsb,
            tc.tile_pool(name="ps", bufs=8, space="PSUM") as ps,
        ):
            ident = constp.tile([D, D], BF16, tag="ident")
            nc.gpsimd.dma_start(ident[:], identb[:])
            ones = constp.tile([D, D], BF16, tag="ones")
            nc.vector.memset(ones[:], 1.0)
            onesf = constp.tile([D, D], F32, tag="onesf")
            nc.vector.memset(onesf[:], 1.0)
            epsb = constp.tile([D, 1], F32, tag="epsb")
            nc.vector.memset(epsb[:], EPS)

            masks = persist.tile([D, 16, 512], BF16, tag="masks")
            nc.gpsimd.dma_start(masks[:], maskt[:].rearrange("t p f -> p t f"))
            wosb = persist.tile([D, QH, HID], BF16, tag="wosb")
            nc.gpsimd.dma_start(wosb[:], wo[:].rearrange("(h p) f -> p h f", p=D))

            qhat = [persist.tile([D, S], BF16, tag=f"qhat{h}", name=f"qhat{h}")
                    for h in range(QH)]
            khat = persist.tile([D, S], BF16, tag="khat")
            vsb = persist.tile([D, NT, D], BF16, tag="vsb")
            outt = [persist.tile([D, S], BF16, tag=f"outt{h}", name=f"outt{h}")
                    for h in range(QH)]

            def emit_proj_rope(j):
                """Projections + rms-norm + rope + v transpose for s block j."""
                js = slice(512 * j, 512 * (j + 1))
                pq = [ps.tile([D, 512], F32, tag="ps", name=f"pq{_h}")
                      for _h in range(QH)]
                pk = ps.tile([D, 512], F32, tag="ps", name="pk")
                pv = ps.tile([D, 512], F32, tag="ps", name="pv")
                for hc in range(NHC):
                    xt_t = xstream.tile([D, 512], BF16, tag="xt", name="xt_t")
                    nc.sync.dma_start(xt_t[:], xt[128 * hc:128 * (hc + 1), js])
                    w_t = wstream.tile([D, (QH + 2) * D], BF16, tag="w", name="w_t")
                    nc.gpsimd.dma_start(w_t[:], wqkv[128 * hc:128 * (hc + 1), :])
                    st = dict(start=(hc == 0), stop=(hc == NHC - 1))
                    for h in range(QH):
                        nc.tensor.matmul(pq[h][:], w_t[:, 128 * h:128 * (h + 1)],
                                         xt_t[:], **st)
                    nc.tensor.matmul(pk[:], w_t[:, QH * D:(QH + 1) * D], xt_t[:], **st)
                    nc.tensor.matmul(pv[:], w_t[:, (QH + 1) * D:], xt_t[:], **st)

                # k first so attention on this block can start earliest
                for h in [QH] + list(range(QH)):
                    if h < QH:
                        psrc, dstt, tdram = pq[h], qhat[h], tabq
                    else:
                        psrc, dstt, tdram = pk, khat, tabk
                    tab = tabstream.tile([D, 2, 512], F32, tag="tab", name="tab")
                    nc.sync.dma_start(tab[:], tdram[:, :, js])
                    # single eviction read frees the PSUM bank quickly
                    qraw = tmp.tile([D, 512], F32, tag="qraw", bufs=3, name="qraw")
                    nc.scalar.copy(qraw[:], psrc[:])
                    sq = tmp.tile([D, 512], BF16, tag="sq", name="sq")
                    nc.scalar.square(sq[:], qraw[:])
                    pss = ps.tile([D, 512], F32, tag="ps", name="pss")
                    nc.tensor.matmul(pss[:], ones[:], sq[:], start=True, stop=True)
                    # r = rsqrt(mean + eps) = exp(-0.5 * ln(sumsq/128 + eps))
                    rbc = tmp.tile([D, 512], F32, tag="rbc", name="rbc")
                    nc.scalar.activation(rbc[:], pss[:], AF.Ln,
                                         bias=epsb[:], scale=1.0 / D)
                    nc.scalar.activation(rbc[:], rbc[:], AF.Exp, bias=0.0, scale=-0.5)
                    t1 = tmp.tile([D, 512], F32, tag="t1", name="t1")
                    nc.vector.tensor_tensor(t1[:], qraw[:], tab[:, 0, :], OP.mult)
                    t2 = tmp.tile([D, 512], F32, tag="t2", name="t2")
                    nc.vector.tensor_tensor(t2[0:64, :], qraw[64:128, :],
                                            tab[64:128, 1, :], OP.mult)
                    nc.vector.tensor_tensor(t2[64:128, :], qraw[0:64, :],
                                            tab[0:64, 1, :], OP.mult)
                    nc.vector.tensor_tensor(t1[:], t1[:], t2[:], OP.add)
                    nc.vector.tensor_tensor(dstt[:, js], t1[:], rbc[:], OP.mult)

                # v: evict + transpose into [t, d] layout
                vt = tmp.tile([D, 512], BF16, tag="vt", name="vt")
                nc.vector.tensor_copy(vt[:], pv[:])
                for c in range(4):
                    pvt = ps.tile([D, D], BF16, tag="ps", name="pvt")
                    nc.tensor.transpose(pvt[:], vt[:, 128 * c:128 * (c + 1)], ident[:])
                    nc.scalar.copy(vsb[:, 4 * j + c, :], pvt[:])

            def emit_attention(j):
                """Attention + o_proj for s block j (k/v tiles 0..4j+3 ready)."""
                js = slice(512 * j, 512 * (j + 1))
                for h in range(QH):
                    po = ps.tile([D, 512], F32, tag="ps", name="po")
                    pd = ps.tile([D, 512], F32, tag="ps", name="pd")
                    ntt = 4 * j + 4
                    for tt in range(ntt):
                        psc = ps.tile([D, 512], F32, tag="ps", name="psc")
                        diag = tt >= 4 * j
                        nc.tensor.matmul(psc[:], khat[:, 128 * tt:128 * (tt + 1)],
                                         qhat[h][:, js], start=True, stop=not diag)
                        if diag:
                            # accumulate the mask on the PE: psc += I.T @ maskT
                            nc.tensor.matmul(psc[:], ident[:], masks[:, tt, :],
                                             start=False, stop=True)
                        ex = expp.tile([D, 512], BF16, tag="ex", name="ex")
                        nc.scalar.activation(ex[:], psc[:], AF.Exp,
                                             bias=0.0, scale=SM_SCALE)
                        st = dict(start=(tt == 0), stop=(tt == ntt - 1))
                        nc.tensor.matmul(po[:], vsb[:, tt, :], ex[:], **st)
                        nc.tensor.matmul(pd[:], ones[:], ex[:], **st)
                    rd = tmp.tile([D, 512], F32, tag="rd", name="rd")
                    nc.scalar.activation(rd[:], pd[:], AF.Ln, bias=0.0, scale=1.0)
                    nc.scalar.activation(rd[:], rd[:], AF.Exp, bias=0.0, scale=-1.0)
                    nc.vector.tensor_tensor(outt[h][:, js], po[:], rd[:], OP.mult)

                for stt in range(4 * j, 4 * j + 4):
                    ss = slice(128 * stt, 128 * (stt + 1))
                    for half in range(2):
                        pb = [ps.tile([D, 512], F32, tag="ps", name=f"pb{_b}")
                              for _b in range(4)]
                        for h in range(QH):
                            for b in range(4):
                                col = 2048 * half + 512 * b
                                nc.tensor.matmul(pb[b][:], outt[h][:, ss],
                                                 wosb[:, h, col:col + 512],
                                                 start=(h == 0), stop=(h == QH - 1))
                        osb = outsb.tile([D, 2048], F32, tag="osb", name="osb")
                        for b in range(4):
                            nc.scalar.copy(osb[:, 512 * b:512 * (b + 1)], pb[b][:])
                        nc.gpsimd.dma_start(out[ss, 2048 * half:2048 * (half + 1)],
                                            osb[:])

            # Software-pipeline by one block: attention(j-1) is emitted after
            # proj(j), so PE always has projection matmuls to run while the
            # rope chain for block j drains on ACT/DVE.
            for j in range(NJ):
                emit_proj_rope(j)
                if j > 0:
                    emit_attention(j - 1)
            emit_attention(NJ - 1)

    _split_multi_waits(nc)
    return nc


_NC_CACHE = None


def _get_program():
    global _NC_CACHE
    if _NC_CACHE is None:
        _NC_CACHE = build_program()
    return _NC_CACHE


def _rope_tables(cos_g, sin_g, w):
    """Pack [D, 2, S]: [:, 0] = cos_g.T * w[d]; [:, 1] = swS where
    swS[d, s] = sign(pair(d)) * sin_g[s, pair(d)] * w[d], i.e. the rotate
    table with halves pre-swapped so t2[lo] = qraw[hi] * swS[hi] etc."""
    half = D // 2
    cw = np.ascontiguousarray((cos_g * w[None, :]).T)
    swS = np.empty((D, S), np.float32)
    swS[:half, :] = (sin_g[:, half:] * w[:half][None, :]).T
    swS[half:, :] = -(sin_g[:, :half] * w[half:][None, :]).T
    return np.ascontiguousarray(np.stack([cw, swS], axis=1))  # [D, 2, S]


def kernel(x, position_ids, cos, sin, attn_mask, Wq, Wk, Wv, Wo, q_norm_w, k_norm_w):
    x = np.asarray(x, np.float32)
    position_ids = np.asarray(position_ids)
    cos_g = np.asarray(cos, np.float32)[position_ids]   # [S, D]
    sin_g = np.asarray(sin, np.float32)[position_ids]
    attn_mask = np.asarray(attn_mask, np.float32)
    Wq = np.asarray(Wq, np.float32)
    Wk = np.asarray(Wk, np.float32)
    Wv = np.asarray(Wv, np.float32)
    Wo = np.asarray(Wo, np.float32)
    qw = np.asarray(q_norm_w, np.float32)
    kw = np.asarray(k_norm_w, np.float32)

    bf = ml_dtypes.bfloat16
    xt = np.ascontiguousarray(x.T).astype(bf)           # [HID, S]

    tabq = _rope_tables(cos_g, sin_g, qw)
    tabk = _rope_tables(cos_g, sin_g, kw)

    # diagonal-band mask tiles of attn_mask.T: tile tt covers scoresT rows
    # 128*tt..128*tt+127 and cols (q positions) 512*(tt//4)..+511
    mT = attn_mask.T
    maskt = np.empty((16, D, 512), np.float32)
    for tt in range(16):
        j = tt // 4
        maskt[tt] = mT[128 * tt:128 * (tt + 1), 512 * j:512 * (j + 1)]
    maskt = maskt.astype(ml_dtypes.bfloat16)

    identb = np.eye(D).astype(bf)

    in_maps = []
    for i in range(NCORES):
        wqkv = np.concatenate([
            Wq[:, QH * D * i:QH * D * (i + 1)],
            Wk[:, D * i:D * (i + 1)],
            Wv[:, D * i:D * (i + 1)],
        ], axis=1).astype(bf)
        in_maps.append({
            "xt": xt,
            "wqkv": np.ascontiguousarray(wqkv),
            "wo": np.ascontiguousarray(Wo[QH * D * i:QH * D * (i + 1), :]).astype(bf),
            "tabq": tabq, "tabk": tabk,
            "maskt": maskt,
            "identb": identb,
        })

    nc = _get_program()
    res = run_bass_kernel_spmd(nc, in_maps, list(range(NCORES)))
    acc = np.zeros((S, HID), np.float32)
    for r in res.results:
        acc += r["out"]
    return acc


# revision 18
# speedup vs baseline: 1.0269x; 1.0250x over previous
"""GQA causal attention block (sparse_attention) on 8 Trainium2 NeuronCores.

Tensor-parallel over heads: core i computes q-heads 4i..4i+3 and kv-head i
(N_KV == n_cores, so each core owns exactly one kv head), plus the matching
row-slice of the o_proj; the 8 partial o_proj outputs are summed on the host.

Layout choice: everything that feeds the PE keeps the contraction dim on
partitions. Projections produce qT/kT/vT [d, s] directly (stationary = weight
chunk, moving = xT), attention scores are computed transposed [t, s]
(stationary = kT slice, moving = qT), PV consumes v [t, d] (stationary) times
exp-scores [t, s] (moving), and o_proj consumes outT [d, s] as stationary.
Softmax denominators come from a ones-matmul (partition-dim reduction on PE,
result pre-broadcast across partitions); reciprocals/rsqrts are computed as
exp(-ln(x)) on the ACT engine to avoid the slow iterative DVE divide.
"""

import sys

sys.path.insert(0, "/opt/trn_rl_repo")

import numpy as np
import ml_dtypes

import concourse.bass as bass
import concourse.mybir as mybir
from concourse import tile
from concourse.vector_clock import ScopedClock, VectorClock
from concourse.bass_utils import run_bass_kernel_spmd

F32 = mybir.dt.float32
BF16 = mybir.dt.bfloat16
AF = mybir.ActivationFunctionType
OP = mybir.AluOpType

S = 2048
HID = 4096
N_HEADS = 32
N_KV = 8
D = 128
NCORES = 8
QH = N_HEADS // NCORES          # q heads per core
EPS = 1e-6
SM_SCALE = float(D) ** -0.5
NJ = S // 512                   # 512-wide s blocks
NHC = HID // 128                # 128-deep contraction chunks
NT = S // 128                   # 128-tall t tiles


class TileContextFixed(tile.TileContext):
    """TileContext whose tail drain emits one sem-wait per Drain instruction.

    The pinned walrus (CoreV3GenImpl setupSyncWait) rejects instructions that
    carry more than one sync-wait command; stock TileContext attaches the
    whole global clock to a single Drain.
    """

    def _drain_and_barrier(self, tick_clock, wait_clock):
        gc = tick_clock.global_clock
        nprocs = len(gc)
        emitted = False
        for proc in range(nprocs):
            tick = gc[proc]
            if tick <= 0:
                continue
            vec = [0] * nprocs
            vec[proc] = tick
            d = self.nc.sync.drain()
            wait_clock.add_sem_waits(d.ins, ScopedClock({None: VectorClock(vec)}))
            emitted = True
        if not emitted:
            self.nc.sync.drain()

        self.nc.all_engine_barrier()
        assert self.sems is not None
        popped = self.nc._tile_sem_poison_stack.pop()
        assert popped is self._sem_poison
        self.nc.clear_and_free_semaphores(list(self.sems.allocated().values()))
        self.nc.all_engine_barrier()


def _split_multi_waits(nc):
    """Hoist all-but-one sem wait of any instruction onto preceding NOPs.

    The pinned walrus rejects instructions with more than one sync-wait
    command; engine streams execute in order, so a same-engine NOP carrying
    the extra waits right before the instruction is equivalent.
    """
    n = 0
    for f in nc.m.functions:
        for bb in f.blocks:
            rebuilt = []
            changed = False
            for inst in bb.instructions:
                si = inst.sync_info
                if si is not None and len(si.on_wait) > 1:
                    waits = list(si.on_wait)
                    for w in waits[:-1]:
                        n += 1
                        nop = mybir.InstNoOp(
                            name=f"I-waitsplit-{n}",
                            engine=inst.engine,
                            sync_info=mybir.SyncInfo(on_wait=[w], on_update=[]),
                            bass_nofuse=True,
                        )
                        nc.register_instruction(nop)
                        rebuilt.append(nop)
                    inst.sync_info = mybir.SyncInfo(
                        on_wait=[waits[-1]], on_update=list(si.on_update)
                    )
                    changed = True
                rebuilt.append(inst)
            if changed:
                bb.instructions = rebuilt


def build_program():
    nc = bass.Bass()

    xt = nc.dram_tensor("xt", [HID, S], BF16, kind="ExternalInput")
    # packed per-core projection weights: [HID, 4*D q | D k | D v]
    wqkv = nc.dram_tensor("wqkv", [HID, (QH + 2) * D], BF16, kind="ExternalInput")
    wo = nc.dram_tensor("wo", [QH * D, HID], BF16, kind="ExternalInput")
    # packed rope tables: [:, 0, :] = cos*w; [:, 1, :] = half-swapped rotate
    # table swS with swS[d] = sign(pair(d))*sin[pair(d)]*w[d], so that
    # rot-half multiplies read both SBUF operands at the same base partition
    tabq = nc.dram_tensor("tabq", [D, 2, S], F32, kind="ExternalInput")
    tabk = nc.dram_tensor("tabk", [D, 2, S], F32, kind="ExternalInput")
    maskt = nc.dram_tensor("maskt", [16, D, 512], BF16, kind="ExternalInput")
    identb = nc.dram_tensor("identb", [D, D], BF16, kind="ExternalInput")
    out = nc.dram_tensor("out", [S, HID], F32, kind="ExternalOutput")

    with TileContextFixed(nc) as tc:
        with (
            tc.tile_pool(name="const", bufs=1) as constp,
            tc.tile_pool(name="persist", bufs=1) as persist,
            tc.tile_pool(name="wstream", bufs=8) as wstream,
            tc.tile_pool(name="xstream", bufs=8) as xstream,
            tc.tile_pool(name="tmp", bufs=2) as tmp,
            tc.tile_pool(name="tabstream", bufs=4) as tabstream,
            tc.tile_pool(name="expp", bufs=4) as expp,
            tc.tile_pool(name="outsb", bufs=2) as outsb,
            tc.tile_pool(name="ps", bufs=8, space="PSUM") as ps,
        ):
            ident = constp.tile([D, D], BF16, tag="ident")
            nc.gpsimd.dma_start(ident[:], identb[:])
            ones = constp.tile([D, D], BF16, tag="ones")
            nc.vector.memset(ones[:], 1.0)
            onesf = constp.tile([D, D], F32, tag="onesf")
            nc.vector.memset(onesf[:], 1.0)
            epsb = constp.tile([D, 1], F32, tag="epsb")
            nc.vector.memset(epsb[:], EPS)

            masks = persist.tile([D, 16, 512], BF16, tag="masks")
            nc.gpsimd.dma_start(masks[:], maskt[:].rearrange("t p f -> p t f"))
            wosb = persist.tile([D, QH, HID], BF16, tag="wosb")
            nc.gpsimd.dma_start(wosb[:], wo[:].rearrange("(h p) f -> p h f", p=D))

            qhat = [persist.tile([D, S], BF16, tag=f"qhat{h}", name=f"qhat{h}")
                    for h in range(QH)]
            khat = persist.tile([D, S], BF16, tag="khat")
            vsb = persist.tile([D, NT, D], BF16, tag="vsb")
            outt = [persist.tile([D, S], BF16, tag=f"outt{h}", name=f"outt{h}")
                    for h in range(QH)]

            def emit_proj(j):
                """Projections for s block j + immediate PSUM evictions.

                Returns the evicted raw projections (SBUF) for the rope stage.
                """
                js = slice(512 * j, 512 * (j + 1))
                pq = [ps.tile([D, 512], F32, tag="ps", name=f"pq{_h}")
                      for _h in range(QH)]
                pk = ps.tile([D, 512], F32, tag="ps", name="pk")
                pv = ps.tile([D, 512], F32, tag="ps", name="pv")
                for hc in range(NHC):
                    xt_t = xstream.tile([D, 512], BF16, tag="xt", name="xt_t")
                    nc.sync.dma_start(xt_t[:], xt[128 * hc:128 * (hc + 1), js])
                    w_t = wstream.tile([D, (QH + 2) * D], BF16, tag="w", name="w_t")
                    nc.gpsimd.dma_start(w_t[:], wqkv[128 * hc:128 * (hc + 1), :])
                    st = dict(start=(hc == 0), stop=(hc == NHC - 1))
                    for h in range(QH):
                        nc.tensor.matmul(pq[h][:], w_t[:, 128 * h:128 * (h + 1)],
                                         xt_t[:], **st)
                    nc.tensor.matmul(pk[:], w_t[:, QH * D:(QH + 1) * D], xt_t[:], **st)
                    nc.tensor.matmul(pv[:], w_t[:, (QH + 1) * D:], xt_t[:], **st)

                # evict all six accumulators right away to free the banks
                qraws = []
                for h in [QH] + list(range(QH)):
                    psrc = pk if h == QH else pq[h]
                    qraw = tmp.tile([D, 512], F32, tag="qraw", bufs=6, name="qraw")
                    nc.scalar.copy(qraw[:], psrc[:])
                    sq = tmp.tile([D, 512], BF16, tag="sq", bufs=6, name="sq")
                    nc.scalar.square(sq[:], qraw[:])
                    qraws.append((h, qraw, sq))
                vt = tmp.tile([D, 512], BF16, tag="vt", name="vt")
                nc.vector.tensor_copy(vt[:], pv[:])
                return qraws, vt

            def emit_rope(j, qraws, vt):
                """RMS-norm + rope (k first) + v transpose for s block j."""
                js = slice(512 * j, 512 * (j + 1))
                for h, qraw, sq in qraws:
                    if h < QH:
                        dstt, tdram = qhat[h], tabq
                    else:
                        dstt, tdram = khat, tabk
                    tab = tabstream.tile([D, 2, 512], F32, tag="tab", name="tab")
                    nc.sync.dma_start(tab[:], tdram[:, :, js])
                    pss = ps.tile([D, 512], F32, tag="ps", name="pss")
                    nc.tensor.matmul(pss[:], ones[:], sq[:], start=True, stop=True)
                    # r = rsqrt(mean + eps) = exp(-0.5 * ln(sumsq/128 + eps))
                    rbc = tmp.tile([D, 512], F32, tag="rbc", name="rbc")
                    nc.scalar.activation(rbc[:], pss[:], AF.Ln,
                                         bias=epsb[:], scale=1.0 / D)
                    nc.scalar.activation(rbc[:], rbc[:], AF.Exp, bias=0.0, scale=-0.5)
                    t1 = tmp.tile([D, 512], F32, tag="t1", name="t1")
                    nc.vector.tensor_tensor(t1[:], qraw[:], tab[:, 0, :], OP.mult)
                    t2 = tmp.tile([D, 512], F32, tag="t2", name="t2")
                    nc.vector.tensor_tensor(t2[0:64, :], qraw[64:128, :],
                                            tab[64:128, 1, :], OP.mult)
                    nc.vector.tensor_tensor(t2[64:128, :], qraw[0:64, :],
                                            tab[0:64, 1, :], OP.mult)
                    nc.vector.tensor_tensor(t1[:], t1[:], t2[:], OP.add)
                    nc.vector.tensor_tensor(dstt[:, js], t1[:], rbc[:], OP.mult)

                for c in range(4):
                    pvt = ps.tile([D, D], BF16, tag="ps", name="pvt")
                    nc.tensor.transpose(pvt[:], vt[:, 128 * c:128 * (c + 1)], ident[:])
                    nc.scalar.copy(vsb[:, 4 * j + c, :], pvt[:])

            def emit_attention(j):
                """Attention + o_proj for s block j (k/v tiles 0..4j+3 ready)."""
                js = slice(512 * j, 512 * (j + 1))
                for h in range(QH):
                    po = ps.tile([D, 512], F32, tag="ps", name="po")
                    pd = ps.tile([D, 512], F32, tag="ps", name="pd")
                    ntt = 4 * j + 4
                    for tt in range(ntt):
                        psc = ps.tile([D, 512], F32, tag="ps", name="psc")
                        diag = tt >= 4 * j
                        nc.tensor.matmul(psc[:], khat[:, 128 * tt:128 * (tt + 1)],
                                         qhat[h][:, js], start=True, stop=not diag)
                        if diag:
                            # accumulate the mask on the PE: psc += I.T @ maskT
                            nc.tensor.matmul(psc[:], ident[:], masks[:, tt, :],
                                             start=False, stop=True)
                        ex = expp.tile([D, 512], BF16, tag="ex", name="ex")
                        nc.scalar.activation(ex[:], psc[:], AF.Exp,
                                             bias=0.0, scale=SM_SCALE)
                        st = dict(start=(tt == 0), stop=(tt == ntt - 1))
                        nc.tensor.matmul(po[:], vsb[:, tt, :], ex[:], **st)
                        nc.tensor.matmul(pd[:], ones[:], ex[:], **st)
                    rd = tmp.tile([D, 512], F32, tag="rd", name="rd")
                    nc.scalar.activation(rd[:], pd[:], AF.Ln, bias=0.0, scale=1.0)
                    nc.scalar.activation(rd[:], rd[:], AF.Exp, bias=0.0, scale=-1.0)
                    nc.vector.tensor_tensor(outt[h][:, js], po[:], rd[:], OP.mult)

                for stt in range(4 * j, 4 * j + 4):
                    ss = slice(128 * stt, 128 * (stt + 1))
                    for half in range(2):
                        pb = [ps.tile([D, 512], F32, tag="ps", name=f"pb{_b}")
                              for _b in range(4)]
                        for h in range(QH):
                            for b in range(4):
                                col = 2048 * half + 512 * b
                                nc.tensor.matmul(pb[b][:], outt[h][:, ss],
                                                 wosb[:, h, col:col + 512],
                                                 start=(h == 0), stop=(h == QH - 1))
                        osb = outsb.tile([D, 2048], F32, tag="osb", name="osb")
                        for b in range(4):
                            nc.scalar.copy(osb[:, 512 * b:512 * (b + 1)], pb[b][:])
                        nc.gpsimd.dma_start(out[ss, 2048 * half:2048 * (half + 1)],
                                            osb[:])

            # Software-pipeline by one block: the PE stream per block is
            # [proj(j) | attention(j-1)+o_proj(j-1) | norm matmuls(j)], so the
            # ACT/DVE rope + norm chains for block j drain while the PE runs
            # attention for block j-1, and vice versa.
            for j in range(NJ):
                qraws, vt = emit_proj(j)
                if j > 0:
                    emit_attention(j - 1)
                emit_rope(j, qraws, vt)
            emit_attention(NJ - 1)

    _split_multi_waits(nc)
    return nc


_NC_CACHE = None


def _get_program():
    global _NC_CACHE
    if _NC_CACHE is None:
        _NC_CACHE = build_program()
    return _NC_CACHE


def _rope_tables(cos_g, sin_g, w):
    """Pack [D, 2, S]: [:, 0] = cos_g.T * w[d]; [:, 1] = swS where
    swS[d, s] = sign(pair(d)) * sin_g[s, pair(d)] * w[d], i.e. the rotate
    table with halves pre-swapped so t2[lo] = qraw[hi] * swS[hi] etc."""
    half = D // 2
    cw = np.ascontiguousarray((cos_g * w[None, :]).T)
    swS = np.empty((D, S), np.float32)
    swS[:half, :] = (sin_g[:, half:] * w[:half][None, :]).T
    swS[half:, :] = -(sin_g[:, :half] * w[half:][None, :]).T
    return np.ascontiguousarray(np.stack([cw, swS], axis=1))  # [D, 2, S]


def kernel(x, position_ids, cos, sin, attn_mask, Wq, Wk, Wv, Wo, q_norm_w, k_norm_w):
    x = np.asarray(x, np.float32)
    position_ids = np.asarray(position_ids)
    cos_g = np.asarray(cos, np.float32)[position_ids]   # [S, D]
    sin_g = np.asarray(sin, np.float32)[position_ids]
    attn_mask = np.asarray(attn_mask, np.float32)
    Wq = np.asarray(Wq, np.float32)
    Wk = np.asarray(Wk, np.float32)
    Wv = np.asarray(Wv, np.float32)
    Wo = np.asarray(Wo, np.float32)
    qw = np.asarray(q_norm_w, np.float32)
    kw = np.asarray(k_norm_w, np.float32)

    bf = ml_dtypes.bfloat16
    xt = np.ascontiguousarray(x.T).astype(bf)           # [HID, S]

    tabq = _rope_tables(cos_g, sin_g, qw)
    tabk = _rope_tables(cos_g, sin_g, kw)

    # diagonal-band mask tiles of attn_mask.T: tile tt covers scoresT rows
    # 128*tt..128*tt+127 and cols (q positions) 512*(tt//4)..+511
    mT = attn_mask.T
    maskt = np.empty((16, D, 512), np.float32)
    for tt in range(16):
        j = tt // 4
        maskt[tt] = mT[128 * tt:128 * (tt + 1), 512 * j:512 * (j + 1)]
    maskt = maskt.astype(ml_dtypes.bfloat16)

    identb = np.eye(D).astype(bf)

    in_maps = []
    for i in range(NCORES):
        wqkv = np.concatenate([
            Wq[:, QH * D * i:QH * D * (i + 1)],
            Wk[:, D * i:D * (i + 1)],
            Wv[:, D * i:D * (i + 1)],
        ], axis=1).astype(bf)
        in_maps.append({
            "xt": xt,
            "wqkv": np.ascontiguousarray(wqkv),
            "wo": np.ascontiguousarray(Wo[QH * D * i:QH * D * (i + 1), :]).astype(bf),
            "tabq": tabq, "tabk": tabk,
            "maskt": maskt,
            "identb": identb,
        })

    nc = _get_program()
    res = run_bass_kernel_spmd(nc, in_maps, list(range(NCORES)))
    acc = np.zeros((S, HID), np.float32)
    for r in res.results:
        acc += r["out"]
    return acc


# revision 19
# speedup vs baseline: 1.0629x; 1.0351x over previous
"""GQA causal attention block (sparse_attention) on 8 Trainium2 NeuronCores.

Tensor-parallel over heads: core i computes q-heads 4i..4i+3 and kv-head i
(N_KV == n_cores, so each core owns exactly one kv head), plus the matching
row-slice of the o_proj; the 8 partial o_proj outputs are summed on the host.

Layout choice: everything that feeds the PE keeps the contraction dim on
partitions. Projections produce qT/kT/vT [d, s] directly (stationary = weight
chunk, moving = xT), attention scores are computed transposed [t, s]
(stationary = kT slice, moving = qT), PV consumes v [t, d] (stationary) times
exp-scores [t, s] (moving), and o_proj consumes outT [d, s] as stationary.
Softmax denominators come from a ones-matmul (partition-dim reduction on PE,
result pre-broadcast across partitions); reciprocals/rsqrts are computed as
exp(-ln(x)) on the ACT engine to avoid the slow iterative DVE divide.
"""

import sys

sys.path.insert(0, "/opt/trn_rl_repo")

import numpy as np
import ml_dtypes

import concourse.bass as bass
import concourse.mybir as mybir
from concourse import tile
from concourse.vector_clock import ScopedClock, VectorClock
from concourse.bass_utils import run_bass_kernel_spmd

F32 = mybir.dt.float32
BF16 = mybir.dt.bfloat16
AF = mybir.ActivationFunctionType
OP = mybir.AluOpType

S = 2048
HID = 4096
N_HEADS = 32
N_KV = 8
D = 128
NCORES = 8
QH = N_HEADS // NCORES          # q heads per core
EPS = 1e-6
SM_SCALE = float(D) ** -0.5
NJ = S // 512                   # 512-wide s blocks
NHC = HID // 128                # 128-deep contraction chunks
NT = S // 128                   # 128-tall t tiles


class TileContextFixed(tile.TileContext):
    """TileContext whose tail drain emits one sem-wait per Drain instruction.

    The pinned walrus (CoreV3GenImpl setupSyncWait) rejects instructions that
    carry more than one sync-wait command; stock TileContext attaches the
    whole global clock to a single Drain.
    """

    def _drain_and_barrier(self, tick_clock, wait_clock):
        gc = tick_clock.global_clock
        nprocs = len(gc)
        emitted = False
        for proc in range(nprocs):
            tick = gc[proc]
            if tick <= 0:
                continue
            vec = [0] * nprocs
            vec[proc] = tick
            d = self.nc.sync.drain()
            wait_clock.add_sem_waits(d.ins, ScopedClock({None: VectorClock(vec)}))
            emitted = True
        if not emitted:
            self.nc.sync.drain()

        self.nc.all_engine_barrier()
        assert self.sems is not None
        popped = self.nc._tile_sem_poison_stack.pop()
        assert popped is self._sem_poison
        self.nc.clear_and_free_semaphores(list(self.sems.allocated().values()))
        self.nc.all_engine_barrier()


def _split_multi_waits(nc):
    """Hoist all-but-one sem wait of any instruction onto preceding NOPs.

    The pinned walrus rejects instructions with more than one sync-wait
    command; engine streams execute in order, so a same-engine NOP carrying
    the extra waits right before the instruction is equivalent.
    """
    n = 0
    for f in nc.m.functions:
        for bb in f.blocks:
            rebuilt = []
            changed = False
            for inst in bb.instructions:
                si = inst.sync_info
                if si is not None and len(si.on_wait) > 1:
                    waits = list(si.on_wait)
                    for w in waits[:-1]:
                        n += 1
                        nop = mybir.InstNoOp(
                            name=f"I-waitsplit-{n}",
                            engine=inst.engine,
                            sync_info=mybir.SyncInfo(on_wait=[w], on_update=[]),
                            bass_nofuse=True,
                        )
                        nc.register_instruction(nop)
                        rebuilt.append(nop)
                    inst.sync_info = mybir.SyncInfo(
                        on_wait=[waits[-1]], on_update=list(si.on_update)
                    )
                    changed = True
                rebuilt.append(inst)
            if changed:
                bb.instructions = rebuilt


def build_program():
    nc = bass.Bass()

    xt = nc.dram_tensor("xt", [HID, S], BF16, kind="ExternalInput")
    # packed per-core projection weights: [HID, 4*D q | D k | D v]
    wqkv = nc.dram_tensor("wqkv", [HID, (QH + 2) * D], BF16, kind="ExternalInput")
    wo = nc.dram_tensor("wo", [QH * D, HID], BF16, kind="ExternalInput")
    # packed rope tables: [:, 0, :] = cos*w; [:, 1, :] = half-swapped rotate
    # table swS with swS[d] = sign(pair(d))*sin[pair(d)]*w[d], so that
    # rot-half multiplies read both SBUF operands at the same base partition
    tabq = nc.dram_tensor("tabq", [D, 2, S], F32, kind="ExternalInput")
    tabk = nc.dram_tensor("tabk", [D, 2, S], F32, kind="ExternalInput")
    maskt = nc.dram_tensor("maskt", [16, D, 512], BF16, kind="ExternalInput")
    identb = nc.dram_tensor("identb", [D, D], BF16, kind="ExternalInput")
    out = nc.dram_tensor("out", [S, HID], F32, kind="ExternalOutput")

    with TileContextFixed(nc) as tc:
        with (
            tc.tile_pool(name="const", bufs=1) as constp,
            tc.tile_pool(name="persist", bufs=1) as persist,
            tc.tile_pool(name="wstream", bufs=8) as wstream,
            tc.tile_pool(name="xstream", bufs=8) as xstream,
            tc.tile_pool(name="tmp", bufs=2) as tmp,
            tc.tile_pool(name="tabstream", bufs=4) as tabstream,
            tc.tile_pool(name="expp", bufs=4) as expp,
            tc.tile_pool(name="outsb", bufs=2) as outsb,
            tc.tile_pool(name="ps", bufs=8, space="PSUM") as ps,
        ):
            ident = constp.tile([D, D], BF16, tag="ident")
            nc.gpsimd.dma_start(ident[:], identb[:])
            ones = constp.tile([D, D], BF16, tag="ones")
            nc.vector.memset(ones[:], 1.0)
            onesf = constp.tile([D, D], F32, tag="onesf")
            nc.vector.memset(onesf[:], 1.0)
            epsb = constp.tile([D, 1], F32, tag="epsb")
            nc.vector.memset(epsb[:], EPS)

            masks = persist.tile([D, 16, 512], BF16, tag="masks")
            nc.gpsimd.dma_start(masks[:], maskt[:].rearrange("t p f -> p t f"))
            wosb = persist.tile([D, QH, HID], BF16, tag="wosb")
            nc.gpsimd.dma_start(wosb[:], wo[:].rearrange("(h p) f -> p h f", p=D))

            qhat = [persist.tile([D, S], BF16, tag=f"qhat{h}", name=f"qhat{h}")
                    for h in range(QH)]
            khat = persist.tile([D, S], BF16, tag="khat")
            vsb = persist.tile([D, NT, D], BF16, tag="vsb")
            outt = [persist.tile([D, S], BF16, tag=f"outt{h}", name=f"outt{h}")
                    for h in range(QH)]

            def emit_proj(j):
                """Projections for s block j + immediate PSUM evictions.

                Returns the evicted raw projections (SBUF) for the rope stage.
                """
                js = slice(512 * j, 512 * (j + 1))
                pq = [ps.tile([D, 512], F32, tag="ps", name=f"pq{_h}")
                      for _h in range(QH)]
                pk = ps.tile([D, 512], F32, tag="ps", name="pk")
                pv = ps.tile([D, 512], F32, tag="ps", name="pv")
                for hc in range(NHC):
                    xt_t = xstream.tile([D, 512], BF16, tag="xt", name="xt_t")
                    nc.sync.dma_start(xt_t[:], xt[128 * hc:128 * (hc + 1), js])
                    w_t = wstream.tile([D, (QH + 2) * D], BF16, tag="w", name="w_t")
                    nc.gpsimd.dma_start(w_t[:], wqkv[128 * hc:128 * (hc + 1), :])
                    st = dict(start=(hc == 0), stop=(hc == NHC - 1))
                    for h in range(QH):
                        nc.tensor.matmul(pq[h][:], w_t[:, 128 * h:128 * (h + 1)],
                                         xt_t[:], **st)
                    nc.tensor.matmul(pk[:], w_t[:, QH * D:(QH + 1) * D], xt_t[:], **st)
                    nc.tensor.matmul(pv[:], w_t[:, (QH + 1) * D:], xt_t[:], **st)

                # evict all six accumulators right away to free the banks
                qraws = []
                for h in [QH] + list(range(QH)):
                    psrc = pk if h == QH else pq[h]
                    qraw = tmp.tile([D, 512], F32, tag="qraw", bufs=6, name="qraw")
                    nc.vector.tensor_copy(qraw[:], psrc[:])
                    sq = tmp.tile([D, 512], BF16, tag="sq", bufs=6, name="sq")
                    nc.vector.tensor_tensor(sq[:], qraw[:], qraw[:], OP.mult)
                    qraws.append((h, qraw, sq))
                vt = tmp.tile([D, 512], BF16, tag="vt", name="vt")
                nc.vector.tensor_copy(vt[:], pv[:])
                return qraws, vt

            def emit_rope(j, qraws, vt):
                """RMS-norm + rope (k first) + v transpose for s block j."""
                js = slice(512 * j, 512 * (j + 1))
                for h, qraw, sq in qraws:
                    if h < QH:
                        dstt, tdram = qhat[h], tabq
                    else:
                        dstt, tdram = khat, tabk
                    tab = tabstream.tile([D, 2, 512], F32, tag="tab", name="tab")
                    nc.sync.dma_start(tab[:], tdram[:, :, js])
                    pss = ps.tile([D, 512], F32, tag="ps", name="pss")
                    nc.tensor.matmul(pss[:], ones[:], sq[:], start=True, stop=True)
                    # r = rsqrt(mean + eps) = exp(-0.5 * ln(sumsq/128 + eps))
                    rbc = tmp.tile([D, 512], F32, tag="rbc", name="rbc")
                    nc.scalar.activation(rbc[:], pss[:], AF.Ln,
                                         bias=epsb[:], scale=1.0 / D)
                    nc.scalar.activation(rbc[:], rbc[:], AF.Exp, bias=0.0, scale=-0.5)
                    t1 = tmp.tile([D, 512], F32, tag="t1", name="t1")
                    nc.vector.tensor_tensor(t1[:], qraw[:], tab[:, 0, :], OP.mult)
                    t2 = tmp.tile([D, 512], F32, tag="t2", name="t2")
                    nc.vector.tensor_tensor(t2[0:64, :], qraw[64:128, :],
                                            tab[64:128, 1, :], OP.mult)
                    nc.vector.tensor_tensor(t2[64:128, :], qraw[0:64, :],
                                            tab[0:64, 1, :], OP.mult)
                    nc.vector.tensor_tensor(t1[:], t1[:], t2[:], OP.add)
                    nc.vector.tensor_tensor(dstt[:, js], t1[:], rbc[:], OP.mult)

                for c in range(4):
                    pvt = ps.tile([D, D], BF16, tag="ps", name="pvt")
                    nc.tensor.transpose(pvt[:], vt[:, 128 * c:128 * (c + 1)], ident[:])
                    nc.scalar.copy(vsb[:, 4 * j + c, :], pvt[:])

            def emit_attention(j):
                """Attention + o_proj for s block j (k/v tiles 0..4j+3 ready)."""
                js = slice(512 * j, 512 * (j + 1))
                for h in range(QH):
                    po = ps.tile([D, 512], F32, tag="ps", name="po")
                    pd = ps.tile([D, 512], F32, tag="ps", name="pd")
                    ntt = 4 * j + 4
                    for tt in range(ntt):
                        psc = ps.tile([D, 512], F32, tag="ps", name="psc")
                        diag = tt >= 4 * j
                        nc.tensor.matmul(psc[:], khat[:, 128 * tt:128 * (tt + 1)],
                                         qhat[h][:, js], start=True, stop=not diag)
                        if diag:
                            # accumulate the mask on the PE: psc += I.T @ maskT
                            nc.tensor.matmul(psc[:], ident[:], masks[:, tt, :],
                                             start=False, stop=True)
                        ex = expp.tile([D, 512], BF16, tag="ex", name="ex")
                        nc.scalar.activation(ex[:], psc[:], AF.Exp,
                                             bias=0.0, scale=SM_SCALE)
                        st = dict(start=(tt == 0), stop=(tt == ntt - 1))
                        nc.tensor.matmul(po[:], vsb[:, tt, :], ex[:], **st)
                        nc.tensor.matmul(pd[:], ones[:], ex[:], **st)
                    rd = tmp.tile([D, 512], F32, tag="rd", name="rd")
                    nc.scalar.activation(rd[:], pd[:], AF.Ln, bias=0.0, scale=1.0)
                    nc.scalar.activation(rd[:], rd[:], AF.Exp, bias=0.0, scale=-1.0)
                    nc.vector.tensor_tensor(outt[h][:, js], po[:], rd[:], OP.mult)

                for stt in range(4 * j, 4 * j + 4):
                    ss = slice(128 * stt, 128 * (stt + 1))
                    for half in range(2):
                        pb = [ps.tile([D, 512], F32, tag="ps", name=f"pb{_b}")
                              for _b in range(4)]
                        for h in range(QH):
                            for b in range(4):
                                col = 2048 * half + 512 * b
                                nc.tensor.matmul(pb[b][:], outt[h][:, ss],
                                                 wosb[:, h, col:col + 512],
                                                 start=(h == 0), stop=(h == QH - 1))
                        osb = outsb.tile([D, 2048], F32, tag="osb", name="osb")
                        for b in range(4):
                            nc.scalar.copy(osb[:, 512 * b:512 * (b + 1)], pb[b][:])
                        nc.gpsimd.dma_start(out[ss, 2048 * half:2048 * (half + 1)],
                                            osb[:])

            # Software-pipeline by one block: the PE stream per block is
            # [proj(j) | attention(j-1)+o_proj(j-1) | norm matmuls(j)], so the
            # ACT/DVE rope + norm chains for block j drain while the PE runs
            # attention for block j-1, and vice versa.
            for j in range(NJ):
                qraws, vt = emit_proj(j)
                if j > 0:
                    emit_attention(j - 1)
                emit_rope(j, qraws, vt)
            emit_attention(NJ - 1)

    _split_multi_waits(nc)
    return nc


_NC_CACHE = None


def _get_program():
    global _NC_CACHE
    if _NC_CACHE is None:
        _NC_CACHE = build_program()
    return _NC_CACHE


def _rope_tables(cos_g, sin_g, w):
    """Pack [D, 2, S]: [:, 0] = cos_g.T * w[d]; [:, 1] = swS where
    swS[d, s] = sign(pair(d)) * sin_g[s, pair(d)] * w[d], i.e. the rotate
    table with halves pre-swapped so t2[lo] = qraw[hi] * swS[hi] etc."""
    half = D // 2
    cw = np.ascontiguousarray((cos_g * w[None, :]).T)
    swS = np.empty((D, S), np.float32)
    swS[:half, :] = (sin_g[:, half:] * w[:half][None, :]).T
    swS[half:, :] = -(sin_g[:, :half] * w[half:][None, :]).T
    return np.ascontiguousarray(np.stack([cw, swS], axis=1))  # [D, 2, S]


def kernel(x, position_ids, cos, sin, attn_mask, Wq, Wk, Wv, Wo, q_norm_w, k_norm_w):
    x = np.asarray(x, np.float32)
    position_ids = np.asarray(position_ids)
    cos_g = np.asarray(cos, np.float32)[position_ids]   # [S, D]
    sin_g = np.asarray(sin, np.float32)[position_ids]
    attn_mask = np.asarray(attn_mask, np.float32)
    Wq = np.asarray(Wq, np.float32)
    Wk = np.asarray(Wk, np.float32)
    Wv = np.asarray(Wv, np.float32)
    Wo = np.asarray(Wo, np.float32)
    qw = np.asarray(q_norm_w, np.float32)
    kw = np.asarray(k_norm_w, np.float32)

    bf = ml_dtypes.bfloat16
    xt = np.ascontiguousarray(x.T).astype(bf)           # [HID, S]

    tabq = _rope_tables(cos_g, sin_g, qw)
    tabk = _rope_tables(cos_g, sin_g, kw)

    # diagonal-band mask tiles of attn_mask.T: tile tt covers scoresT rows
    # 128*tt..128*tt+127 and cols (q positions) 512*(tt//4)..+511
    mT = attn_mask.T
    maskt = np.empty((16, D, 512), np.float32)
    for tt in range(16):
        j = tt // 4
        maskt[tt] = mT[128 * tt:128 * (tt + 1), 512 * j:512 * (j + 1)]
    maskt = maskt.astype(ml_dtypes.bfloat16)

    identb = np.eye(D).astype(bf)

    in_maps = []
    for i in range(NCORES):
        wqkv = np.concatenate([
            Wq[:, QH * D * i:QH * D * (i + 1)],
            Wk[:, D * i:D * (i + 1)],
            Wv[:, D * i:D * (i + 1)],
        ], axis=1).astype(bf)
        in_maps.append({
            "xt": xt,
            "wqkv": np.ascontiguousarray(wqkv),
            "wo": np.ascontiguousarray(Wo[QH * D * i:QH * D * (i + 1), :]).astype(bf),
            "tabq": tabq, "tabk": tabk,
            "maskt": maskt,
            "identb": identb,
        })

    nc = _get_program()
    res = run_bass_kernel_spmd(nc, in_maps, list(range(NCORES)))
    acc = np.zeros((S, HID), np.float32)
    for r in res.results:
        acc += r["out"]
    return acc


# revision 20
# speedup vs baseline: 1.1059x; 1.0404x over previous
"""GQA causal attention block (sparse_attention) on 8 Trainium2 NeuronCores.

Tensor-parallel over heads: core i computes q-heads 4i..4i+3 and kv-head i
(N_KV == n_cores, so each core owns exactly one kv head), plus the matching
row-slice of the o_proj; the 8 partial o_proj outputs are summed on the host.

Layout choice: everything that feeds the PE keeps the contraction dim on
partitions. Projections produce qT/kT/vT [d, s] directly (stationary = weight
chunk, moving = xT), attention scores are computed transposed [t, s]
(stationary = kT slice, moving = qT), PV consumes v [t, d] (stationary) times
exp-scores [t, s] (moving), and o_proj consumes outT [d, s] as stationary.
Softmax denominators come from a ones-matmul (partition-dim reduction on PE,
result pre-broadcast across partitions); reciprocals/rsqrts are computed as
exp(-ln(x)) on the ACT engine to avoid the slow iterative DVE divide.
"""

import sys

sys.path.insert(0, "/opt/trn_rl_repo")

import numpy as np
import ml_dtypes

import concourse.bass as bass
import concourse.mybir as mybir
from concourse import tile
from concourse.vector_clock import ScopedClock, VectorClock
from concourse.bass_utils import run_bass_kernel_spmd

F32 = mybir.dt.float32
BF16 = mybir.dt.bfloat16
AF = mybir.ActivationFunctionType
OP = mybir.AluOpType

S = 2048
HID = 4096
N_HEADS = 32
N_KV = 8
D = 128
NCORES = 8
QH = N_HEADS // NCORES          # q heads per core
EPS = 1e-6
SM_SCALE = float(D) ** -0.5
NJ = S // 512                   # 512-wide s blocks
NHC = HID // 128                # 128-deep contraction chunks
NT = S // 128                   # 128-tall t tiles


class TileContextFixed(tile.TileContext):
    """TileContext whose tail drain emits one sem-wait per Drain instruction.

    The pinned walrus (CoreV3GenImpl setupSyncWait) rejects instructions that
    carry more than one sync-wait command; stock TileContext attaches the
    whole global clock to a single Drain.
    """

    def _drain_and_barrier(self, tick_clock, wait_clock):
        gc = tick_clock.global_clock
        nprocs = len(gc)
        emitted = False
        for proc in range(nprocs):
            tick = gc[proc]
            if tick <= 0:
                continue
            vec = [0] * nprocs
            vec[proc] = tick
            d = self.nc.sync.drain()
            wait_clock.add_sem_waits(d.ins, ScopedClock({None: VectorClock(vec)}))
            emitted = True
        if not emitted:
            self.nc.sync.drain()

        self.nc.all_engine_barrier()
        assert self.sems is not None
        popped = self.nc._tile_sem_poison_stack.pop()
        assert popped is self._sem_poison
        self.nc.clear_and_free_semaphores(list(self.sems.allocated().values()))
        self.nc.all_engine_barrier()


def _split_multi_waits(nc):
    """Hoist all-but-one sem wait of any instruction onto preceding NOPs.

    The pinned walrus rejects instructions with more than one sync-wait
    command; engine streams execute in order, so a same-engine NOP carrying
    the extra waits right before the instruction is equivalent.
    """
    n = 0
    for f in nc.m.functions:
        for bb in f.blocks:
            rebuilt = []
            changed = False
            for inst in bb.instructions:
                si = inst.sync_info
                if si is not None and len(si.on_wait) > 1:
                    waits = list(si.on_wait)
                    for w in waits[:-1]:
                        n += 1
                        nop = mybir.InstNoOp(
                            name=f"I-waitsplit-{n}",
                            engine=inst.engine,
                            sync_info=mybir.SyncInfo(on_wait=[w], on_update=[]),
                            bass_nofuse=True,
                        )
                        nc.register_instruction(nop)
                        rebuilt.append(nop)
                    inst.sync_info = mybir.SyncInfo(
                        on_wait=[waits[-1]], on_update=list(si.on_update)
                    )
                    changed = True
                rebuilt.append(inst)
            if changed:
                bb.instructions = rebuilt


def build_program():
    nc = bass.Bass()

    xt = nc.dram_tensor("xt", [HID, S], BF16, kind="ExternalInput")
    # packed per-core projection weights: [HID, 4*D q | D k | D v]
    wqkv = nc.dram_tensor("wqkv", [HID, (QH + 2) * D], BF16, kind="ExternalInput")
    wo = nc.dram_tensor("wo", [QH * D, HID], BF16, kind="ExternalInput")
    # packed rope tables: [:, 0, :] = cos*w; [:, 1, :] = half-swapped rotate
    # table swS with swS[d] = sign(pair(d))*sin[pair(d)]*w[d], so that
    # rot-half multiplies read both SBUF operands at the same base partition
    tabq = nc.dram_tensor("tabq", [D, 2, S], F32, kind="ExternalInput")
    tabk = nc.dram_tensor("tabk", [D, 2, S], F32, kind="ExternalInput")
    maskt = nc.dram_tensor("maskt", [16, D, 512], BF16, kind="ExternalInput")
    identb = nc.dram_tensor("identb", [D, D], BF16, kind="ExternalInput")
    out = nc.dram_tensor("out", [S, HID], F32, kind="ExternalOutput")

    with TileContextFixed(nc) as tc:
        with (
            tc.tile_pool(name="const", bufs=1) as constp,
            tc.tile_pool(name="persist", bufs=1) as persist,
            tc.tile_pool(name="wstream", bufs=8) as wstream,
            tc.tile_pool(name="xstream", bufs=8) as xstream,
            tc.tile_pool(name="tmp", bufs=2) as tmp,
            tc.tile_pool(name="tabstream", bufs=4) as tabstream,
            tc.tile_pool(name="expp", bufs=4) as expp,
            tc.tile_pool(name="outsb", bufs=2) as outsb,
            tc.tile_pool(name="ps", bufs=8, space="PSUM") as ps,
        ):
            ident = constp.tile([D, D], BF16, tag="ident")
            nc.gpsimd.dma_start(ident[:], identb[:])
            ones = constp.tile([D, D], BF16, tag="ones")
            nc.vector.memset(ones[:], 1.0)
            onesf = constp.tile([D, D], F32, tag="onesf")
            nc.vector.memset(onesf[:], 1.0)
            epsb = constp.tile([D, 1], F32, tag="epsb")
            nc.vector.memset(epsb[:], EPS)

            masks = persist.tile([D, 16, 512], BF16, tag="masks")
            wosb = persist.tile([D, QH, HID], BF16, tag="wosb")

            qhat = [persist.tile([D, S], BF16, tag=f"qhat{h}", name=f"qhat{h}")
                    for h in range(QH)]
            khat = persist.tile([D, S], BF16, tag="khat")
            vsb = persist.tile([D, NT, D], BF16, tag="vsb")
            outt = [persist.tile([D, S], BF16, tag=f"outt{h}", name=f"outt{h}")
                    for h in range(QH)]

            def emit_proj(j):
                """Projections for s block j + immediate PSUM evictions.

                Returns the evicted raw projections (SBUF) for the rope stage.
                """
                js = slice(512 * j, 512 * (j + 1))
                pq = [ps.tile([D, 512], F32, tag="ps", name=f"pq{_h}")
                      for _h in range(QH)]
                pk = ps.tile([D, 512], F32, tag="ps", name="pk")
                pv = ps.tile([D, 512], F32, tag="ps", name="pv")
                for hc in range(NHC):
                    xt_t = xstream.tile([D, 512], BF16, tag="xt", name="xt_t")
                    nc.sync.dma_start(xt_t[:], xt[128 * hc:128 * (hc + 1), js])
                    w_t = wstream.tile([D, (QH + 2) * D], BF16, tag="w", name="w_t")
                    nc.gpsimd.dma_start(w_t[:], wqkv[128 * hc:128 * (hc + 1), :])
                    st = dict(start=(hc == 0), stop=(hc == NHC - 1))
                    for h in range(QH):
                        nc.tensor.matmul(pq[h][:], w_t[:, 128 * h:128 * (h + 1)],
                                         xt_t[:], **st)
                    nc.tensor.matmul(pk[:], w_t[:, QH * D:(QH + 1) * D], xt_t[:], **st)
                    nc.tensor.matmul(pv[:], w_t[:, (QH + 1) * D:], xt_t[:], **st)

                # evict all six accumulators right away to free the banks
                qraws = []
                for h in [QH] + list(range(QH)):
                    psrc = pk if h == QH else pq[h]
                    qraw = tmp.tile([D, 512], F32, tag="qraw", bufs=6, name="qraw")
                    nc.vector.tensor_copy(qraw[:], psrc[:])
                    sq = tmp.tile([D, 512], BF16, tag="sq", bufs=6, name="sq")
                    nc.vector.tensor_tensor(sq[:], qraw[:], qraw[:], OP.mult)
                    qraws.append((h, qraw, sq))
                vt = tmp.tile([D, 512], BF16, tag="vt", name="vt")
                nc.vector.tensor_copy(vt[:], pv[:])
                return qraws, vt

            def emit_rope(j, qraws, vt):
                """RMS-norm + rope (k first) + v transpose for s block j."""
                js = slice(512 * j, 512 * (j + 1))
                for h, qraw, sq in qraws:
                    if h < QH:
                        dstt, tdram = qhat[h], tabq
                    else:
                        dstt, tdram = khat, tabk
                    tab = tabstream.tile([D, 2, 512], F32, tag="tab", name="tab")
                    nc.sync.dma_start(tab[:], tdram[:, :, js])
                    pss = ps.tile([D, 512], F32, tag="ps", name="pss")
                    nc.tensor.matmul(pss[:], ones[:], sq[:], start=True, stop=True)
                    # r = rsqrt(mean + eps) = exp(-0.5 * ln(sumsq/128 + eps))
                    rbc = tmp.tile([D, 512], F32, tag="rbc", name="rbc")
                    nc.scalar.activation(rbc[:], pss[:], AF.Ln,
                                         bias=epsb[:], scale=1.0 / D)
                    nc.scalar.activation(rbc[:], rbc[:], AF.Exp, bias=0.0, scale=-0.5)
                    t1 = tmp.tile([D, 512], F32, tag="t1", name="t1")
                    nc.vector.tensor_tensor(t1[:], qraw[:], tab[:, 0, :], OP.mult)
                    t2 = tmp.tile([D, 512], F32, tag="t2", name="t2")
                    nc.vector.tensor_tensor(t2[0:64, :], qraw[64:128, :],
                                            tab[64:128, 1, :], OP.mult)
                    nc.vector.tensor_tensor(t2[64:128, :], qraw[0:64, :],
                                            tab[0:64, 1, :], OP.mult)
                    nc.vector.tensor_tensor(t1[:], t1[:], t2[:], OP.add)
                    nc.vector.tensor_tensor(dstt[:, js], t1[:], rbc[:], OP.mult)

                for c in range(4):
                    pvt = ps.tile([D, D], BF16, tag="ps", name="pvt")
                    nc.tensor.transpose(pvt[:], vt[:, 128 * c:128 * (c + 1)], ident[:])
                    nc.scalar.copy(vsb[:, 4 * j + c, :], pvt[:])

            def emit_attention(j):
                """Attention + o_proj for s block j (k/v tiles 0..4j+3 ready)."""
                js = slice(512 * j, 512 * (j + 1))
                for h in range(QH):
                    po = ps.tile([D, 512], F32, tag="ps", name="po")
                    pd = ps.tile([D, 512], F32, tag="ps", name="pd")
                    ntt = 4 * j + 4
                    pending = []
                    for tt in range(ntt):
                        psc = ps.tile([D, 512], F32, tag="ps", name="psc")
                        diag = tt >= 4 * j
                        nc.tensor.matmul(psc[:], khat[:, 128 * tt:128 * (tt + 1)],
                                         qhat[h][:, js], start=True, stop=not diag)
                        if diag:
                            # accumulate the mask on the PE: psc += I.T @ maskT
                            nc.tensor.matmul(psc[:], ident[:], masks[:, tt, :],
                                             start=False, stop=True)
                        ex = expp.tile([D, 512], BF16, tag="ex", name="ex")
                        nc.scalar.activation(ex[:], psc[:], AF.Exp,
                                             bias=0.0, scale=SM_SCALE)
                        pending.append((tt, ex))
                        # keep the PE two score tiles ahead of the exp chain
                        if len(pending) > 2:
                            ptt, pex = pending.pop(0)
                            stf = dict(start=(ptt == 0), stop=(ptt == ntt - 1))
                            nc.tensor.matmul(po[:], vsb[:, ptt, :], pex[:], **stf)
                            nc.tensor.matmul(pd[:], ones[:], pex[:], **stf)
                    for ptt, pex in pending:
                        stf = dict(start=(ptt == 0), stop=(ptt == ntt - 1))
                        nc.tensor.matmul(po[:], vsb[:, ptt, :], pex[:], **stf)
                        nc.tensor.matmul(pd[:], ones[:], pex[:], **stf)
                    rd = tmp.tile([D, 512], F32, tag="rd", name="rd")
                    nc.scalar.activation(rd[:], pd[:], AF.Ln, bias=0.0, scale=1.0)
                    nc.scalar.activation(rd[:], rd[:], AF.Exp, bias=0.0, scale=-1.0)
                    nc.vector.tensor_tensor(outt[h][:, js], po[:], rd[:], OP.mult)

                for stt in range(4 * j, 4 * j + 4):
                    ss = slice(128 * stt, 128 * (stt + 1))
                    for half in range(2):
                        pb = [ps.tile([D, 512], F32, tag="ps", name=f"pb{_b}")
                              for _b in range(4)]
                        for h in range(QH):
                            for b in range(4):
                                col = 2048 * half + 512 * b
                                nc.tensor.matmul(pb[b][:], outt[h][:, ss],
                                                 wosb[:, h, col:col + 512],
                                                 start=(h == 0), stop=(h == QH - 1))
                        osb = outsb.tile([D, 2048], F32, tag="osb", name="osb")
                        for b in range(4):
                            eng = nc.scalar if b % 2 == 0 else nc.vector
                            if b % 2 == 0:
                                nc.scalar.copy(osb[:, 512 * b:512 * (b + 1)], pb[b][:])
                            else:
                                nc.vector.tensor_copy(osb[:, 512 * b:512 * (b + 1)],
                                                      pb[b][:])
                        nc.gpsimd.dma_start(out[ss, 2048 * half:2048 * (half + 1)],
                                            osb[:])

            # Software-pipeline by one block: the PE stream per block is
            # [proj(j) | attention(j-1)+o_proj(j-1) | norm matmuls(j)], so the
            # ACT/DVE rope + norm chains for block j drain while the PE runs
            # attention for block j-1, and vice versa.
            for j in range(NJ):
                qraws, vt = emit_proj(j)
                if j == 0:
                    nc.gpsimd.dma_start(masks[:],
                                        maskt[:].rearrange("t p f -> p t f"))
                    nc.gpsimd.dma_start(wosb[:],
                                        wo[:].rearrange("(h p) f -> p h f", p=D))
                if j > 0:
                    emit_attention(j - 1)
                emit_rope(j, qraws, vt)
            emit_attention(NJ - 1)

    _split_multi_waits(nc)
    return nc


_NC_CACHE = None


def _get_program():
    global _NC_CACHE
    if _NC_CACHE is None:
        _NC_CACHE = build_program()
    return _NC_CACHE


def _rope_tables(cos_g, sin_g, w):
    """Pack [D, 2, S]: [:, 0] = cos_g.T * w[d]; [:, 1] = swS where
    swS[d, s] = sign(pair(d)) * sin_g[s, pair(d)] * w[d], i.e. the rotate
    table with halves pre-swapped so t2[lo] = qraw[hi] * swS[hi] etc."""
    half = D // 2
    cw = np.ascontiguousarray((cos_g * w[None, :]).T)
    swS = np.empty((D, S), np.float32)
    swS[:half, :] = (sin_g[:, half:] * w[:half][None, :]).T
    swS[half:, :] = -(sin_g[:, :half] * w[half:][None, :]).T
    return np.ascontiguousarray(np.stack([cw, swS], axis=1))  # [D, 2, S]


def kernel(x, position_ids, cos, sin, attn_mask, Wq, Wk, Wv, Wo, q_norm_w, k_norm_w):
    x = np.asarray(x, np.float32)
    position_ids = np.asarray(position_ids)
    cos_g = np.asarray(cos, np.float32)[position_ids]   # [S, D]
    sin_g = np.asarray(sin, np.float32)[position_ids]
    attn_mask = np.asarray(attn_mask, np.float32)
    Wq = np.asarray(Wq, np.float32)
    Wk = np.asarray(Wk, np.float32)
    Wv = np.asarray(Wv, np.float32)
    Wo = np.asarray(Wo, np.float32)
    qw = np.asarray(q_norm_w, np.float32)
    kw = np.asarray(k_norm_w, np.float32)

    bf = ml_dtypes.bfloat16
    xt = np.ascontiguousarray(x.T).astype(bf)           # [HID, S]

    tabq = _rope_tables(cos_g, sin_g, qw)
    tabk = _rope_tables(cos_g, sin_g, kw)

    # diagonal-band mask tiles of attn_mask.T: tile tt covers scoresT rows
    # 128*tt..128*tt+127 and cols (q positions) 512*(tt//4)..+511
    mT = attn_mask.T
    maskt = np.empty((16, D, 512), np.float32)
    for tt in range(16):
        j = tt // 4
        maskt[tt] = mT[128 * tt:128 * (tt + 1), 512 * j:512 * (j + 1)]
    maskt = maskt.astype(ml_dtypes.bfloat16)

    identb = np.eye(D).astype(bf)

    in_maps = []
    for i in range(NCORES):
        wqkv = np.concatenate([
            Wq[:, QH * D * i:QH * D * (i + 1)],
            Wk[:, D * i:D * (i + 1)],
            Wv[:, D * i:D * (i + 1)],
        ], axis=1).astype(bf)
        in_maps.append({
            "xt": xt,
            "wqkv": np.ascontiguousarray(wqkv),
            "wo": np.ascontiguousarray(Wo[QH * D * i:QH * D * (i + 1), :]).astype(bf),
            "tabq": tabq, "tabk": tabk,
            "maskt": maskt,
            "identb": identb,
        })

    nc = _get_program()
    res = run_bass_kernel_spmd(nc, in_maps, list(range(NCORES)))
    acc = np.zeros((S, HID), np.float32)
    for r in res.results:
        acc += r["out"]
    return acc


# revision 21
# speedup vs baseline: 1.1111x; 1.0047x over previous
"""GQA causal attention block (sparse_attention) on 8 Trainium2 NeuronCores.

Tensor-parallel over heads: core i computes q-heads 4i..4i+3 and kv-head i
(N_KV == n_cores, so each core owns exactly one kv head), plus the matching
row-slice of the o_proj; the 8 partial o_proj outputs are summed on the host.

Layout choice: everything that feeds the PE keeps the contraction dim on
partitions. Projections produce qT/kT/vT [d, s] directly (stationary = weight
chunk, moving = xT), attention scores are computed transposed [t, s]
(stationary = kT slice, moving = qT), PV consumes v [t, d] (stationary) times
exp-scores [t, s] (moving), and o_proj consumes outT [d, s] as stationary.
Softmax denominators come from a ones-matmul (partition-dim reduction on PE,
result pre-broadcast across partitions); reciprocals/rsqrts are computed as
exp(-ln(x)) on the ACT engine to avoid the slow iterative DVE divide.
"""

import sys

sys.path.insert(0, "/opt/trn_rl_repo")

import numpy as np
import ml_dtypes

import concourse.bass as bass
import concourse.mybir as mybir
from concourse import tile
from concourse.vector_clock import ScopedClock, VectorClock
from concourse.bass_utils import run_bass_kernel_spmd

F32 = mybir.dt.float32
BF16 = mybir.dt.bfloat16
AF = mybir.ActivationFunctionType
OP = mybir.AluOpType

S = 2048
HID = 4096
N_HEADS = 32
N_KV = 8
D = 128
NCORES = 8
QH = N_HEADS // NCORES          # q heads per core
EPS = 1e-6
SM_SCALE = float(D) ** -0.5
NJ = S // 512                   # 512-wide s blocks
NHC = HID // 128                # 128-deep contraction chunks
NT = S // 128                   # 128-tall t tiles


class TileContextFixed(tile.TileContext):
    """TileContext whose tail drain emits one sem-wait per Drain instruction.

    The pinned walrus (CoreV3GenImpl setupSyncWait) rejects instructions that
    carry more than one sync-wait command; stock TileContext attaches the
    whole global clock to a single Drain.
    """

    def _drain_and_barrier(self, tick_clock, wait_clock):
        gc = tick_clock.global_clock
        nprocs = len(gc)
        emitted = False
        for proc in range(nprocs):
            tick = gc[proc]
            if tick <= 0:
                continue
            vec = [0] * nprocs
            vec[proc] = tick
            d = self.nc.sync.drain()
            wait_clock.add_sem_waits(d.ins, ScopedClock({None: VectorClock(vec)}))
            emitted = True
        if not emitted:
            self.nc.sync.drain()

        self.nc.all_engine_barrier()
        assert self.sems is not None
        popped = self.nc._tile_sem_poison_stack.pop()
        assert popped is self._sem_poison
        self.nc.clear_and_free_semaphores(list(self.sems.allocated().values()))
        self.nc.all_engine_barrier()


def _split_multi_waits(nc):
    """Hoist all-but-one sem wait of any instruction onto preceding NOPs.

    The pinned walrus rejects instructions with more than one sync-wait
    command; engine streams execute in order, so a same-engine NOP carrying
    the extra waits right before the instruction is equivalent.
    """
    n = 0
    for f in nc.m.functions:
        for bb in f.blocks:
            rebuilt = []
            changed = False
            for inst in bb.instructions:
                si = inst.sync_info
                if si is not None and len(si.on_wait) > 1:
                    waits = list(si.on_wait)
                    for w in waits[:-1]:
                        n += 1
                        nop = mybir.InstNoOp(
                            name=f"I-waitsplit-{n}",
                            engine=inst.engine,
                            sync_info=mybir.SyncInfo(on_wait=[w], on_update=[]),
                            bass_nofuse=True,
                        )
                        nc.register_instruction(nop)
                        rebuilt.append(nop)
                    inst.sync_info = mybir.SyncInfo(
                        on_wait=[waits[-1]], on_update=list(si.on_update)
                    )
                    changed = True
                rebuilt.append(inst)
            if changed:
                bb.instructions = rebuilt


def build_program():
    nc = bass.Bass()

    xt = nc.dram_tensor("xt", [HID, S], BF16, kind="ExternalInput")
    # packed per-core projection weights: [HID, 4*D q | D k | D v]
    wqkv = nc.dram_tensor("wqkv", [HID, (QH + 2) * D], BF16, kind="ExternalInput")
    wo = nc.dram_tensor("wo", [QH * D, HID], BF16, kind="ExternalInput")
    # packed rope tables: [:, 0, :] = cos*w; [:, 1, :] = half-swapped rotate
    # table swS with swS[d] = sign(pair(d))*sin[pair(d)]*w[d], so that
    # rot-half multiplies read both SBUF operands at the same base partition
    tabq = nc.dram_tensor("tabq", [D, 2, S], F32, kind="ExternalInput")
    tabk = nc.dram_tensor("tabk", [D, 2, S], F32, kind="ExternalInput")
    maskt = nc.dram_tensor("maskt", [16, D, 512], BF16, kind="ExternalInput")
    identb = nc.dram_tensor("identb", [D, D], BF16, kind="ExternalInput")
    out = nc.dram_tensor("out", [S, HID], F32, kind="ExternalOutput")

    with TileContextFixed(nc) as tc:
        with (
            tc.tile_pool(name="const", bufs=1) as constp,
            tc.tile_pool(name="persist", bufs=1) as persist,
            tc.tile_pool(name="wstream", bufs=8) as wstream,
            tc.tile_pool(name="xstream", bufs=8) as xstream,
            tc.tile_pool(name="tmp", bufs=2) as tmp,
            tc.tile_pool(name="tabstream", bufs=4) as tabstream,
            tc.tile_pool(name="expp", bufs=4) as expp,
            tc.tile_pool(name="outsb", bufs=2) as outsb,
            tc.tile_pool(name="ps", bufs=8, space="PSUM") as ps,
        ):
            ident = constp.tile([D, D], BF16, tag="ident")
            nc.gpsimd.dma_start(ident[:], identb[:])
            ones = constp.tile([D, D], BF16, tag="ones")
            nc.vector.memset(ones[:], 1.0)
            onesf = constp.tile([D, D], F32, tag="onesf")
            nc.vector.memset(onesf[:], 1.0)
            epsb = constp.tile([D, 1], F32, tag="epsb")
            nc.vector.memset(epsb[:], EPS)

            masks = persist.tile([D, 16, 512], BF16, tag="masks")
            wosb = persist.tile([D, QH, HID], BF16, tag="wosb")

            qhat = [persist.tile([D, S], BF16, tag=f"qhat{h}", name=f"qhat{h}")
                    for h in range(QH)]
            khat = persist.tile([D, S], BF16, tag="khat")
            vsb = persist.tile([D, NT, D], BF16, tag="vsb")
            outt = [persist.tile([D, S], BF16, tag=f"outt{h}", name=f"outt{h}")
                    for h in range(QH)]

            def emit_proj(j):
                """Projections for s block j + immediate PSUM evictions.

                Returns the evicted raw projections (SBUF) for the rope stage.
                """
                js = slice(512 * j, 512 * (j + 1))
                pq = [ps.tile([D, 512], F32, tag="ps", name=f"pq{_h}")
                      for _h in range(QH)]
                pk = ps.tile([D, 512], F32, tag="ps", name="pk")
                pv = ps.tile([D, 512], F32, tag="ps", name="pv")
                for hc in range(NHC):
                    xt_t = xstream.tile([D, 512], BF16, tag="xt", name="xt_t")
                    nc.sync.dma_start(xt_t[:], xt[128 * hc:128 * (hc + 1), js])
                    w_t = wstream.tile([D, (QH + 2) * D], BF16, tag="w", name="w_t")
                    nc.gpsimd.dma_start(w_t[:], wqkv[128 * hc:128 * (hc + 1), :])
                    st = dict(start=(hc == 0), stop=(hc == NHC - 1))
                    for h in range(QH):
                        nc.tensor.matmul(pq[h][:], w_t[:, 128 * h:128 * (h + 1)],
                                         xt_t[:], **st)
                    nc.tensor.matmul(pk[:], w_t[:, QH * D:(QH + 1) * D], xt_t[:], **st)
                    nc.tensor.matmul(pv[:], w_t[:, (QH + 1) * D:], xt_t[:], **st)

                # evict all six accumulators right away to free the banks
                qraws = []
                for h in [QH] + list(range(QH)):
                    psrc = pk if h == QH else pq[h]
                    qraw = tmp.tile([D, 512], F32, tag="qraw", bufs=6, name="qraw")
                    nc.vector.tensor_copy(qraw[:], psrc[:])
                    sq = tmp.tile([D, 512], BF16, tag="sq", bufs=6, name="sq")
                    nc.vector.tensor_tensor(sq[:], qraw[:], qraw[:], OP.mult)
                    qraws.append((h, qraw, sq))
                vt = tmp.tile([D, 512], BF16, tag="vt", name="vt")
                nc.vector.tensor_copy(vt[:], pv[:])
                return qraws, vt

            def emit_rope(j, qraws, vt):
                """RMS-norm + rope (k first) + v transpose for s block j."""
                js = slice(512 * j, 512 * (j + 1))
                for h, qraw, sq in qraws:
                    if h < QH:
                        dstt, tdram = qhat[h], tabq
                    else:
                        dstt, tdram = khat, tabk
                    tab = tabstream.tile([D, 2, 512], F32, tag="tab", name="tab")
                    nc.sync.dma_start(tab[:], tdram[:, :, js])
                    pss = ps.tile([D, 512], F32, tag="ps", name="pss")
                    nc.tensor.matmul(pss[:], ones[:], sq[:], start=True, stop=True)
                    # r = rsqrt(mean + eps) = exp(-0.5 * ln(sumsq/128 + eps))
                    rbc = tmp.tile([D, 512], F32, tag="rbc", name="rbc")
                    nc.scalar.activation(rbc[:], pss[:], AF.Ln,
                                         bias=epsb[:], scale=1.0 / D)
                    nc.scalar.activation(rbc[:], rbc[:], AF.Exp, bias=0.0, scale=-0.5)
                    t1 = tmp.tile([D, 512], F32, tag="t1", name="t1")
                    nc.vector.tensor_tensor(t1[:], qraw[:], tab[:, 0, :], OP.mult)
                    t2 = tmp.tile([D, 512], F32, tag="t2", name="t2")
                    nc.vector.tensor_tensor(t2[0:64, :], qraw[64:128, :],
                                            tab[64:128, 1, :], OP.mult)
                    nc.vector.tensor_tensor(t2[64:128, :], qraw[0:64, :],
                                            tab[0:64, 1, :], OP.mult)
                    nc.vector.tensor_tensor(t1[:], t1[:], t2[:], OP.add)
                    nc.vector.tensor_tensor(dstt[:, js], t1[:], rbc[:], OP.mult)

                for c in range(4):
                    pvt = ps.tile([D, D], BF16, tag="ps", name="pvt")
                    nc.tensor.transpose(pvt[:], vt[:, 128 * c:128 * (c + 1)], ident[:])
                    nc.scalar.copy(vsb[:, 4 * j + c, :], pvt[:])

            def emit_attention(j):
                """Attention + o_proj for s block j (k/v tiles 0..4j+3 ready)."""
                js = slice(512 * j, 512 * (j + 1))
                for h in range(QH):
                    po = ps.tile([D, 512], F32, tag="ps", name="po")
                    pd = ps.tile([D, 512], F32, tag="ps", name="pd")
                    ntt = 4 * j + 4
                    pending = []
                    for tt in range(ntt):
                        psc = ps.tile([D, 512], F32, tag="ps", name="psc")
                        diag = tt >= 4 * j
                        nc.tensor.matmul(psc[:], khat[:, 128 * tt:128 * (tt + 1)],
                                         qhat[h][:, js], start=True, stop=not diag)
                        if diag:
                            # accumulate the mask on the PE: psc += I.T @ maskT
                            nc.tensor.matmul(psc[:], ident[:], masks[:, tt, :],
                                             start=False, stop=True)
                        ex = expp.tile([D, 512], BF16, tag="ex", name="ex")
                        nc.scalar.activation(ex[:], psc[:], AF.Exp,
                                             bias=0.0, scale=SM_SCALE)
                        pending.append((tt, ex))
                        # keep the PE two score tiles ahead of the exp chain
                        if len(pending) > 2:
                            ptt, pex = pending.pop(0)
                            stf = dict(start=(ptt == 0), stop=(ptt == ntt - 1))
                            nc.tensor.matmul(po[:], vsb[:, ptt, :], pex[:], **stf)
                            nc.tensor.matmul(pd[:], ones[:], pex[:], **stf)
                    for ptt, pex in pending:
                        stf = dict(start=(ptt == 0), stop=(ptt == ntt - 1))
                        nc.tensor.matmul(po[:], vsb[:, ptt, :], pex[:], **stf)
                        nc.tensor.matmul(pd[:], ones[:], pex[:], **stf)
                    rd = tmp.tile([D, 512], F32, tag="rd", name="rd")
                    nc.scalar.activation(rd[:], pd[:], AF.Ln, bias=0.0, scale=1.0)
                    nc.scalar.activation(rd[:], rd[:], AF.Exp, bias=0.0, scale=-1.0)
                    nc.vector.tensor_tensor(outt[h][:, js], po[:], rd[:], OP.mult)

                for stt in range(4 * j, 4 * j + 4):
                    ss = slice(128 * stt, 128 * (stt + 1))
                    for half in range(2):
                        pb = [ps.tile([D, 512], F32, tag="ps", name=f"pb{_b}")
                              for _b in range(4)]
                        for h in range(QH):
                            for b in range(4):
                                col = 2048 * half + 512 * b
                                nc.tensor.matmul(pb[b][:], outt[h][:, ss],
                                                 wosb[:, h, col:col + 512],
                                                 start=(h == 0), stop=(h == QH - 1))
                        osb = outsb.tile([D, 2048], F32, tag="osb", name="osb")
                        for b in range(4):
                            eng = nc.scalar if b % 2 == 0 else nc.vector
                            if b % 2 == 0:
                                nc.scalar.copy(osb[:, 512 * b:512 * (b + 1)], pb[b][:])
                            else:
                                nc.vector.tensor_copy(osb[:, 512 * b:512 * (b + 1)],
                                                      pb[b][:])
                        nc.gpsimd.dma_start(out[ss, 2048 * half:2048 * (half + 1)],
                                            osb[:])

            # Software-pipeline by one block: the PE stream per block is
            # [proj(j) | attention(j-1)+o_proj(j-1) | norm matmuls(j)], so the
            # ACT/DVE rope + norm chains for block j drain while the PE runs
            # attention for block j-1, and vice versa.
            for j in range(NJ):
                qraws, vt = emit_proj(j)
                if j == 1:
                    nc.gpsimd.dma_start(masks[:],
                                        maskt[:].rearrange("t p f -> p t f"))
                    nc.gpsimd.dma_start(wosb[:],
                                        wo[:].rearrange("(h p) f -> p h f", p=D))
                if j > 0:
                    emit_attention(j - 1)
                emit_rope(j, qraws, vt)
            emit_attention(NJ - 1)

    _split_multi_waits(nc)
    return nc


_NC_CACHE = None


def _get_program():
    global _NC_CACHE
    if _NC_CACHE is None:
        _NC_CACHE = build_program()
    return _NC_CACHE


def _rope_tables(cos_g, sin_g, w):
    """Pack [D, 2, S]: [:, 0] = cos_g.T * w[d]; [:, 1] = swS where
    swS[d, s] = sign(pair(d)) * sin_g[s, pair(d)] * w[d], i.e. the rotate
    table with halves pre-swapped so t2[lo] = qraw[hi] * swS[hi] etc."""
    half = D // 2
    cw = np.ascontiguousarray((cos_g * w[None, :]).T)
    swS = np.empty((D, S), np.float32)
    swS[:half, :] = (sin_g[:, half:] * w[:half][None, :]).T
    swS[half:, :] = -(sin_g[:, :half] * w[half:][None, :]).T
    return np.ascontiguousarray(np.stack([cw, swS], axis=1))  # [D, 2, S]


def kernel(x, position_ids, cos, sin, attn_mask, Wq, Wk, Wv, Wo, q_norm_w, k_norm_w):
    x = np.asarray(x, np.float32)
    position_ids = np.asarray(position_ids)
    cos_g = np.asarray(cos, np.float32)[position_ids]   # [S, D]
    sin_g = np.asarray(sin, np.float32)[position_ids]
    attn_mask = np.asarray(attn_mask, np.float32)
    Wq = np.asarray(Wq, np.float32)
    Wk = np.asarray(Wk, np.float32)
    Wv = np.asarray(Wv, np.float32)
    Wo = np.asarray(Wo, np.float32)
    qw = np.asarray(q_norm_w, np.float32)
    kw = np.asarray(k_norm_w, np.float32)

    bf = ml_dtypes.bfloat16
    xt = np.ascontiguousarray(x.T).astype(bf)           # [HID, S]

    tabq = _rope_tables(cos_g, sin_g, qw)
    tabk = _rope_tables(cos_g, sin_g, kw)

    # diagonal-band mask tiles of attn_mask.T: tile tt covers scoresT rows
    # 128*tt..128*tt+127 and cols (q positions) 512*(tt//4)..+511
    mT = attn_mask.T
    maskt = np.empty((16, D, 512), np.float32)
    for tt in range(16):
        j = tt // 4
        maskt[tt] = mT[128 * tt:128 * (tt + 1), 512 * j:512 * (j + 1)]
    maskt = maskt.astype(ml_dtypes.bfloat16)

    identb = np.eye(D).astype(bf)

    in_maps = []
    for i in range(NCORES):
        wqkv = np.concatenate([
            Wq[:, QH * D * i:QH * D * (i + 1)],
            Wk[:, D * i:D * (i + 1)],
            Wv[:, D * i:D * (i + 1)],
        ], axis=1).astype(bf)
        in_maps.append({
            "xt": xt,
            "wqkv": np.ascontiguousarray(wqkv),
            "wo": np.ascontiguousarray(Wo[QH * D * i:QH * D * (i + 1), :]).astype(bf),
            "tabq": tabq, "tabk": tabk,
            "maskt": maskt,
            "identb": identb,
        })

    nc = _get_program()
    res = run_bass_kernel_spmd(nc, in_maps, list(range(NCORES)))
    acc = np.zeros((S, HID), np.float32)
    for r in res.results:
        acc += r["out"]
    return acc
